# revision 13
# baseline (speedup 1.0000x reference)
"""Multi-head causal attention (B=4, T=2048, E=1024, H=16, D=64) on 8 trn2
NeuronCores via Bass/Tile.

Sharding: core c handles batch b = c//2 and heads [half*8, half*8+8), half =
c%2. Each core computes its 8 heads' attention and a partial output
projection Y^T = Wp_slice^T-contraction over its heads; the host sums the two
half partials per batch, transposes, and adds the bias.

On-device layout is "transposed": activations are [feature, token] so every
matmul contracts over the partition dim. Softmax denominators come from a
ones-column appended to the stationary V operand (M=65 matmuls); masking is
applied block-wise (128x128) with patterns derived from the actual mask input
at build time. No max-subtraction is needed: scores are ~N(0, 0.083^2).

Scheduling: the kernel is software-pipelined around the ACT-engine exp, which
is the per-block rate limiter during attention. Dense PE work (the next
t-tile's projections and the previous tile's output projection) is split into
single-matmul "filler" closures that are popped between attention i-groups to
fill what would otherwise be PE stalls. A dummy-matmul warmup at t=0 flips
the PE HAM clock gate to 8/8 before real work lands. Softmax normalization is
per head-pair: reciprocal_approx_fast on the psum row, then a K=1 float32r
broadcast matmul.
"""
import numpy as np
import ml_dtypes
from contextlib import ExitStack

import concourse.bass as bass
import concourse.mybir as mybir
import concourse.tile as tile
from concourse.bass_utils import run_bass_kernel_spmd
from concourse.vector_clock import ScopedClock

BF16 = mybir.dt.bfloat16
F32 = mybir.dt.float32
F32R = mybir.dt.float32r
NPBF16 = ml_dtypes.bfloat16

B, T, E, H, D = 4, 2048, 1024, 16, 64
HPC = 8            # heads per core
DC = HPC * D       # 512: stacked head dim per core
TJ = 512           # t tile (matmul free dim)
NJ = T // TJ       # 4
SI = 128           # s tile (psum partition dim)
NSI = T // SI      # 16
EC = E // 128      # 8 e-chunks
NP = HPC // 2      # 4 head pairs
_DUMMY_FILL = True
_DROP_OWN_WAITS = False

# ---------------------------------------------------------------------------
# Workarounds for this walrus build: at most ONE sync wait per instruction.
# ---------------------------------------------------------------------------
_PATCHED = False


def _patched_drain_and_barrier(self, tick_clock, wait_clock):
    drain_inst = self.nc.sync.drain(fusable=False)
    wait_clock.add_sem_waits(
        drain_inst.ins, ScopedClock({None: tick_clock.global_clock})
    )
    si = drain_inst.ins.sync_info
    if si is not None and len(si.on_wait) > 1:
        waits = list(si.on_wait)
        drain_inst.ins.sync_info = mybir.SyncInfo(
            on_wait=waits[:1], on_update=list(si.on_update)
        )
        for ofs in range(1, len(waits)):
            extra = self.nc.sync.drain(fusable=False)
            extra.ins.sync_info = mybir.SyncInfo(
                on_wait=waits[ofs : ofs + 1], on_update=[]
            )
    self.nc.all_engine_barrier()
    assert self.sems is not None
    popped = self.nc._tile_sem_poison_stack.pop()
    assert popped is self._sem_poison
    self.nc.clear_and_free_semaphores(list(self.sems.allocated().values()))
    self.nc.all_engine_barrier()


def _install_patches():
    global _PATCHED
    if _PATCHED:
        return
    tile.TileContext._drain_and_barrier = _patched_drain_and_barrier
    _PATCHED = True


def _make_carrier(nc, engine, wait):
    """Wait-only EventSemaphore on `engine` (cheap: ~70ns, no pipe flush)."""
    ev = mybir.InstEventSemaphore(name=f"W-{nc.next_id()}", ins=[], outs=[])
    ev.engine = engine
    ev.sync_info = mybir.SyncInfo(on_wait=[wait], on_update=[])
    return ev


_ENGINE_SEM = {
    "EngineType.PE": "PE",
    "EngineType.DVE": "DVE",
    "EngineType.Activation": "Activation",
    "EngineType.SP": "SP",
    "EngineType.Pool": "Pool",
}
# engines with in-order issue AND in-order completion for these inst types:
# a wait on the engine's own completion sem is redundant. Ldweights excluded
# (the PE reorder window pulls it ahead of in-flight matmuls).
_DROPPABLE = (
    "InstMatmult", "InstActivation", "InstTensorTensor", "InstTensorCopy",
    "InstTensorReduce", "InstMemset", "InstReciprocal", "InstDMACopy",
    "InstCopyPredicated", "InstTensorScalarPtr", "InstTensorScalar",
    "InstCast", "InstDveOp", "InstCustomDve",
)


def _split_multi_waits(nc):
    for bbw in list(nc.bb_map.values()):
        bb = bbw.bb
        insts = bb.instructions
        if not any(
            getattr(i, "sync_info", None) is not None and len(i.sync_info.on_wait) > 1
            for i in insts
        ):
            continue
        out = []
        for inst in insts:
            si = getattr(inst, "sync_info", None)
            waits = list(si.on_wait) if si is not None else []
            if len(waits) > 1 and _DROP_OWN_WAITS:
                own = _ENGINE_SEM.get(str(inst.engine))
                tn = type(inst).__name__
                if own is not None and tn.startswith(_DROPPABLE):
                    waits = [
                        w for w in waits
                        if w.ant_name.rsplit("_", 1)[0] != own
                    ] or waits[-1:]
            if len(waits) > 1:
                for w in waits[:-1]:
                    out.append(_make_carrier(nc, inst.engine, w))
                waits = waits[-1:]
            if si is not None and list(si.on_wait) != waits:
                inst.sync_info = mybir.SyncInfo(
                    on_wait=waits, on_update=list(si.on_update)
                )
            out.append(inst)
        insts[:] = out


# ---------------------------------------------------------------------------
# Mask analysis (host side, 128x128 blocks).
# ---------------------------------------------------------------------------
def _classify_mask(mask):
    """mask: [T, T] bool, mask[t, s]=True means masked (score -> -inf).

    Returns (btab, patterns): btab[i][jj] in {'skip', 'dense', int u};
    patterns[u] is a [128,128] bf16 multiplier in [s, t] orientation."""
    nb = T // 128
    m = np.asarray(mask, dtype=bool)
    patterns = []
    index = {}
    btab = [[None] * nb for _ in range(nb)]
    for i in range(nb):          # s block
        for jj in range(nb):     # t block
            sub = m[jj * 128 : (jj + 1) * 128, i * 128 : (i + 1) * 128]  # [t, s]
            if sub.all():
                btab[i][jj] = "skip"
            elif not sub.any():
                btab[i][jj] = "dense"
            else:
                pat = (~sub).T.astype(NPBF16)  # [s, t] multiplier
                key = pat.tobytes()
                if key not in index:
                    index[key] = len(patterns)
                    patterns.append(pat)
                btab[i][jj] = index[key]
    if not patterns:
        patterns.append(np.ones((128, 128), NPBF16))
    return btab, np.stack(patterns)


# ---------------------------------------------------------------------------
# Kernel builder (SPMD program, identical on all 8 cores).
# ---------------------------------------------------------------------------
def _build(btab, n_pat):
    nc = bass.Bass()
    qT = nc.declare_dram_parameter("qT", [E, T], BF16, isOutput=False)
    kT = nc.declare_dram_parameter("kT", [E, T], BF16, isOutput=False)
    vT = nc.declare_dram_parameter("vT", [E, T], BF16, isOutput=False)
    wq = nc.declare_dram_parameter("wq", [E, DC], BF16, isOutput=False)
    wk = nc.declare_dram_parameter("wk", [E, DC], BF16, isOutput=False)
    wv = nc.declare_dram_parameter("wv", [E, DC], BF16, isOutput=False)
    wpT = nc.declare_dram_parameter("wpT", [DC, E], BF16, isOutput=False)
    pat = nc.declare_dram_parameter("pat", [n_pat * 128, 128], BF16, isOutput=False)
    selp2 = nc.declare_dram_parameter("selp2", [2, 128], BF16, isOutput=False)
    yT = nc.declare_dram_parameter("yT", [E, T], F32, isOutput=True)

    with ExitStack() as ctx:
        tc = ctx.enter_context(tile.TileContext(nc))
        # SBUF pools
        consts = ctx.enter_context(tc.tile_pool(name="consts", bufs=1))
        streams = ctx.enter_context(tc.tile_pool(name="streams", bufs=1))
        acts = ctx.enter_context(tc.tile_pool(name="acts", bufs=1))
        work = ctx.enter_context(tc.tile_pool(name="work", bufs=1))
        # PSUM pools
        psA = ctx.enter_context(tc.tile_pool(name="psA", bufs=1, space="PSUM"))
        psB = ctx.enter_context(tc.tile_pool(name="psB", bufs=1, space="PSUM"))

        # ---- constants ----
        wq_sb = [consts.tile([128, DC], BF16, tag=f"wq{e}", name=f"wq{e}", bufs=1) for e in range(EC)]
        wk_sb = [consts.tile([128, DC], BF16, tag=f"wk{e}", name=f"wk{e}", bufs=1) for e in range(EC)]
        wv_sb = [consts.tile([128, DC], BF16, tag=f"wv{e}", name=f"wv{e}", bufs=1) for e in range(EC)]
        wp_sb = [consts.tile([128, E], BF16, tag=f"wp{p}", name=f"wp{p}", bufs=1) for p in range(NP)]
        pat_sb = [consts.tile([128, 128], BF16, tag=f"pat{u}", name=f"pat{u}", bufs=1) for u in range(n_pat)]
        selp2_sb = consts.tile([2, 128], BF16, tag="selp2", name="selp2", bufs=1)
        dummy_sb = consts.tile([128, TJ], BF16, tag="dummy", name="dummy", bufs=1)

        # ---- warmup: flip the PE HAM clock gate to 8/8 while DMAs land ----
        nc.vector.memset(dummy_sb[:], 0.0)
        warm_ps = psA.tile([128, TJ], F32, tag="mm512", bufs=2, name="warm")
        for _ in range(10):
            nc.tensor.matmul(
                warm_ps[:], dummy_sb[:, 0:128], dummy_sb[:], start=True, stop=True
            )
        warm_n = [0]

        def dummy_fill(n_mms, ncols=TJ):
            """Keep the PE busy/warm across a known stall with throwaway MMs."""
            if not _DUMMY_FILL:
                return
            warm_n[0] += 1
            ps = psA.tile([128, TJ], F32, tag="mm512", bufs=2,
                          name=f"warmf{warm_n[0]}")
            for _ in range(n_mms):
                nc.tensor.matmul(
                    ps[:, 0:ncols], dummy_sb[:, 0:128], dummy_sb[:, 0:ncols],
                    start=True, stop=True,
                )

        # ---- persistent activations ----
        xq_sb = [acts.tile([128, T], BF16, tag=f"xq{p}", name=f"xq{p}", bufs=1) for p in range(NP)]
        xk_sb = [acts.tile([128, T], BF16, tag=f"xk{p}", name=f"xk{p}", bufs=1) for p in range(NP)]
        # xv tiles: per s-tile, heads laid out as 8 x (64 cols xv | 1 col ones)
        xv_sb = [acts.tile([128, HPC * 65], BF16, tag=f"xv{i}", name=f"xv{i}", bufs=1) for i in range(NSI)]
        for i in range(NSI):
            nc.vector.memset(
                xv_sb[i][:].rearrange("p (h x) -> p h x", x=65)[:, :, 64:65], 1.0
            )
        osc_sb_all = [
            [acts.tile([128, TJ], BF16, tag=f"osc{p}_{jj}", name=f"osc{p}_{jj}", bufs=1)
             for p in range(NP)]
            for jj in range(2)
        ]

        EXP = mybir.ActivationFunctionType.Exp
        stream_tiles = {}

        def issue_dma(j):
            jt = slice(j * TJ, (j + 1) * TJ)
            qs = [streams.tile([128, TJ], BF16, tag=f"qs{e}", name=f"qs{e}_{j}", bufs=2) for e in range(EC)]
            ks = [streams.tile([128, TJ], BF16, tag=f"ks{e}", name=f"ks{e}_{j}", bufs=2) for e in range(EC)]
            vs = [streams.tile([128, TJ], BF16, tag=f"vs{e}", name=f"vs{e}_{j}", bufs=2) for e in range(EC)]
            for e in range(EC):
                er = slice(e * 128, (e + 1) * 128)
                nc.sync.dma_start(out=qs[e][:], in_=qT[er, jt])
                if j == 0:
                    nc.sync.dma_start(out=wq_sb[e][:], in_=wq[er, :])
            for e in range(EC):
                er = slice(e * 128, (e + 1) * 128)
                nc.sync.dma_start(out=ks[e][:], in_=kT[er, jt])
                if j == 0:
                    nc.sync.dma_start(out=wk_sb[e][:], in_=wk[er, :])
            for e in range(EC):
                er = slice(e * 128, (e + 1) * 128)
                nc.sync.dma_start(out=vs[e][:], in_=vT[er, jt])
                if j == 0:
                    nc.sync.dma_start(out=wv_sb[e][:], in_=wv[er, :])
            if j == 0:
                for u in range(n_pat):
                    nc.sync.dma_start(out=pat_sb[u][:], in_=pat[u * 128 : (u + 1) * 128, :])
                for p in range(NP):
                    nc.sync.dma_start(out=wp_sb[p][:], in_=wpT[p * 128 : (p + 1) * 128, :])
                nc.sync.dma_start(out=selp2_sb[:], in_=selp2[:])
            stream_tiles[j] = (qs, ks, vs)

        def proj_qk_fillers(j, pairs):
            """xq/xk projection for t-tile j, given pairs: one closure per MM."""
            qs, ks, _ = stream_tiles[j]
            jt = slice(j * TJ, (j + 1) * TJ)
            fillers = []
            for p in pairs:
                pc = slice(p * 128, (p + 1) * 128)
                for src, Wsb, dst in ((qs, wq_sb, xq_sb), (ks, wk_sb, xk_sb)):
                    cell = {}
                    for e in range(EC):
                        def f(cell=cell, src=src, Wsb=Wsb, dst=dst, e=e, p=p, pc=pc, jt=jt):
                            if e == 0:
                                cell["ps"] = psA.tile([128, TJ], F32, tag="mm512", bufs=2,
                                                      name=f"pqk_{j}_{p}")
                            nc.tensor.matmul(
                                cell["ps"][:], Wsb[e][:, pc],
                                src[e][:], start=(e == 0), stop=(e == EC - 1),
                            )
                            if e == EC - 1:
                                nc.vector.tensor_copy(dst[p][:, jt], cell["ps"][:])
                        fillers.append(f)
            return fillers

        def proj_v_fillers(j):
            """xv projection for t-tile j: one closure per MM."""
            _, _, vs = stream_tiles[j]
            fillers = []
            for loc in range(4):
                si = 4 * j + loc
                cell = {}
                for e in range(EC):
                    def f(cell=cell, e=e, loc=loc, si=si, vs=vs):
                        if e == 0:
                            cell["ps"] = psA.tile([128, DC], F32, tag="mm512", bufs=2,
                                                  name=f"pv_{si}")
                        nc.tensor.matmul(
                            cell["ps"][:], vs[e][:, loc * 128 : (loc + 1) * 128],
                            wv_sb[e][:], start=(e == 0), stop=(e == EC - 1),
                        )
                        if e == EC - 1:
                            nc.vector.tensor_copy(
                                xv_sb[si][:].rearrange("p (h x) -> p h x", x=65)[:, :, 0:64],
                                cell["ps"][:].rearrange("p (h d) -> p h d", h=HPC),
                            )
                    fillers.append(f)
            return fillers

        def y_fillers(j, osc_tiles):
            """output projection partial Y^T[:, j-tile]: one closure per MM."""
            jt = slice(j * TJ, (j + 1) * TJ)
            fillers = []
            for m in range(EC):
                cell = {}
                for p in range(NP):
                    def f(cell=cell, m=m, p=p, jt=jt, osc_tiles=osc_tiles, j=j):
                        if p == 0:
                            cell["ps"] = psA.tile([128, TJ], F32, tag="mm512", bufs=2,
                                                  name=f"y_{m}_{j}")
                        nc.tensor.matmul(
                            cell["ps"][:], wp_sb[p][:, m * 128 : (m + 1) * 128],
                            osc_tiles[p][:], start=(p == 0), stop=(p == NP - 1),
                        )
                        if p == NP - 1:
                            y_sb = work.tile([128, TJ], F32, tag="y", bufs=2,
                                             name=f"ysb_{m}_{j}")
                            nc.vector.tensor_copy(y_sb[:], cell["ps"][:])
                            nc.sync.dma_start(out=yT[m * 128 : (m + 1) * 128, jt], in_=y_sb[:])
                    fillers.append(f)
            return fillers

        def emit_av(j, p, o_ps, touched, ii, i, types, u, c0, n_i):
            """AV matmuls for s-block i of pair p (both heads)."""
            for hh in range(2):
                h = 2 * p + hh
                uo = hh * TJ
                runs = []  # (bl0, bl1, src_ap)
                bl = c0
                while bl < 4:
                    if types[bl] == "dense":
                        b2 = bl
                        while b2 + 1 < 4 and types[b2 + 1] == "dense":
                            b2 += 1
                        runs.append((bl, b2 + 1,
                                     u[:, uo + bl * 128 : uo + (b2 + 1) * 128]))
                        bl = b2 + 1
                    elif types[bl] == "skip":
                        bl += 1
                    else:
                        mt = work.tile([128, 128], BF16, tag="mfix", bufs=4)
                        nc.vector.tensor_mul(
                            mt[:], u[:, uo + bl * 128 : uo + (bl + 1) * 128],
                            pat_sb[types[bl]][:],
                        )
                        runs.append((bl, bl + 1, mt[:]))
                        bl += 1
                lhs_v = xv_sb[i][:, h * 65 : h * 65 + 65]
                for ri, (b0, b1, src) in enumerate(runs):
                    first = all(not touched[hh][b] for b in range(b0, b1))
                    assert first == any(
                        not touched[hh][b] for b in range(b0, b1)
                    ), "mask blocks: mixed touch state inside a run"
                    last = (ii == n_i - 1) and (ri == len(runs) - 1)
                    nc.tensor.matmul(
                        o_ps[hh][:, b0 * 128 : b1 * 128],
                        lhs_v, src,
                        start=first, stop=last,
                        skip_group_check=True,
                    )
                    for b in range(b0, b1):
                        touched[hh][b] = True

        def run_attention(j, filler_q):
            jt = slice(j * TJ, (j + 1) * TJ)
            osc_sb = osc_sb_all[j % 2]
            ivals = []
            for i in range(NSI):
                types = [btab[i][4 * j + bl] for bl in range(4)]
                if all(t == "skip" for t in types):
                    continue
                ivals.append((i, types))
            n_i = len(ivals)
            tails_out = []
            groups_total = max(1, NP * n_i)
            rate = len(filler_q) / groups_total
            state = {"acc": 0.0, "popped": 0, "g": 0}

            def pops(p):
                state["g"] += 1
                state["acc"] += rate
                if j == 0 and state["g"] <= 6:
                    return  # let the j=1 stream DMAs land first
                want = min(int(state["acc"]) - state["popped"], 3)
                if j == NJ - 1 and p == 0:
                    want = max(want, 2)
                for _ in range(want):
                    if filler_q:
                        filler_q.pop(0)()
                        state["popped"] += 1

            for p in range(NP):
                o_ps = [
                    psB.tile([65, TJ], F32, tag=f"ops{hh}", name=f"ops{hh}_{p}_{j}", bufs=1)
                    for hh in range(2)
                ]
                touched = [[False] * 4, [False] * 4]
                prev = None
                for ii, (i, types) in enumerate(ivals):
                    c0 = next(bl for bl in range(4) if types[bl] != "skip")
                    # scores for both heads: row-tiled concurrent K=64 matmuls
                    st = psA.tile([128, 2 * TJ], F32, tag="st", bufs=2)
                    for hh in range(2):
                        hr = slice(hh * 64, (hh + 1) * 64)
                        nc.tensor.matmul(
                            st[:, hh * TJ + c0 * 128 : (hh + 1) * TJ],
                            xk_sb[p][hr, i * 128 : (i + 1) * 128],
                            xq_sb[p][hr, jt][:, c0 * 128 : TJ],
                            start=True, stop=True,
                        )
                    u = work.tile([128, 2 * TJ], BF16, tag="u", bufs=4)
                    nc.scalar.activation(
                        u[:].rearrange("p (g c) -> p g c", g=2)[:, :, c0 * 128 : TJ],
                        st[:].rearrange("p (g c) -> p g c", g=2)[:, :, c0 * 128 : TJ],
                        EXP, scale=1.0 / 32.0,
                    )
                    pops(p)
                    if prev is not None:
                        emit_av(j, p, o_ps, touched, *prev, n_i)
                    prev = (ii, i, types, u, c0)
                emit_av(j, p, o_ps, touched, *prev, n_i)

                # per-pair softmax denominators: scatter the two psum
                # ones-rows to [32, 32] (cheap 32-elem/lane reciprocal),
                # gather back, broadcast with a K=2 selector matmul.
                rcp2 = work.tile([32, 32], F32, tag="rcp2", bufs=4,
                                 name=f"rcp2_{p}_{j}")
                for hh in range(2):
                    rsb = work.tile([1, TJ], F32, tag="rsb", bufs=4)
                    nc.vector.tensor_copy(rsb[:], o_ps[hh][64:65, :])
                    nc.sync.dma_start(
                        out=rcp2[16 * hh : 16 * hh + 16, :], in_=rsb[:]
                    )
                rrc2 = work.tile([32, 32], F32, tag="rrc2", bufs=4,
                                 name=f"rrc2_{p}_{j}")
                nc.vector.reciprocal(rrc2[:], rcp2[:])
                rrb2 = work.tile([32, 32], BF16, tag="rrb2", bufs=4,
                                 name=f"rrb2_{p}_{j}")
                nc.vector.tensor_copy(rrb2[:], rrc2[:])
                rrp = work.tile([2, TJ], BF16, tag="rrp", bufs=4,
                                name=f"rrp_{p}_{j}")
                nc.sync.dma_start(out=rrp[:], in_=rrb2[:])
                osb = work.tile([128, TJ], BF16, tag="osb", bufs=8,
                                name=f"osb_{p}_{j}")
                for hh in range(2):
                    nc.vector.tensor_copy(
                        osb[hh * 64 : (hh + 1) * 64, :], o_ps[hh][0:64, :]
                    )

                def tail_p(p=p, rrp=rrp, osb=osb, osc=osc_sb[p], j=j):
                    rb_ps = psA.tile([128, TJ], F32, tag="mm512", bufs=2,
                                     name=f"rb_{p}_{j}")
                    nc.tensor.matmul(
                        rb_ps[:], selp2_sb[:], rrp[:], start=True, stop=True,
                    )
                    nc.vector.tensor_mul(osc[:], osb[:], rb_ps[:])

                if j == NJ - 1 and p == NP - 1:
                    last_tail[0] = tail_p
                elif j == NJ - 1:
                    filler_q.append(tail_p)
                else:
                    tails_out.append(tail_p)

            # drain leftovers
            while filler_q:
                filler_q.pop(0)()
            return tails_out

        # ------------------- main flow -------------------
        issue_dma(0)
        last_tail = [None]
        for idx, f in enumerate(proj_qk_fillers(0, range(NP)) + proj_v_fillers(0)):
            f()
            if idx % 8 == 7:
                dummy_fill(2, 256)

        pending = []
        deferred = []
        for j in range(NJ):
            if j + 1 < NJ:
                issue_dma(j + 1)
            filler_q = []
            filler_q += deferred
            deferred = []
            filler_q += pending
            if j + 1 < NJ:
                if j + 1 < NJ - 1:
                    filler_q += proj_qk_fillers(j + 1, range(NP))
                    filler_q += proj_v_fillers(j + 1)
                else:
                    # final tile: keep some projection work as filler for the
                    # filler-starved last attention phase
                    filler_q += proj_qk_fillers(j + 1, [0, 1])
                    deferred = proj_v_fillers(j + 1) + proj_qk_fillers(j + 1, [2, 3])
            tails = run_attention(j, filler_q)
            pending = tails + y_fillers(j, osc_sb_all[j % 2])
        dummy_fill(12)
        last_tail[0]()
        for f in pending:
            f()

    _split_multi_waits(nc)
    return nc


_SELP2 = np.zeros((2, 128), NPBF16)
_SELP2[0, 0:64] = 1.0
_SELP2[1, 64:128] = 1.0

_CACHE = {}


def _get_program(mask):
    key = np.asarray(mask, dtype=bool).tobytes()
    prog = _CACHE.get(key)
    if prog is None:
        _install_patches()
        btab, patterns = _classify_mask(mask)
        nc = _build(btab, len(patterns))
        prog = (nc, patterns)
        _CACHE[key] = prog
    return prog


def _prepare(k, q, v, mask, Wk, Wq, Wv, Wp):
    """Build (cached) the SPMD program and the 8 per-core input maps."""
    k = np.asarray(k, np.float32)
    q = np.asarray(q, np.float32)
    v = np.asarray(v, np.float32)
    Wk = np.asarray(Wk, np.float32)
    Wq = np.asarray(Wq, np.float32)
    Wv = np.asarray(Wv, np.float32)
    Wp = np.asarray(Wp, np.float32)

    nc, patterns = _get_program(mask)
    patflat = np.ascontiguousarray(patterns.reshape(-1, 128))

    def tr(x):  # [T, E] f32 -> [E, T] bf16 contiguous
        return np.ascontiguousarray(x.astype(NPBF16).T)

    def wcat(W, half):  # [H, E, D] -> [E, 512] bf16 for this half's 8 heads
        return np.ascontiguousarray(
            W[half * HPC : (half + 1) * HPC].transpose(1, 0, 2).reshape(E, DC)
        ).astype(NPBF16)

    in_maps = []
    for c in range(8):
        b, half = divmod(c, 2)
        off = half * DC
        in_maps.append(
            {
                "qT": tr(q[b]),
                "kT": tr(k[b]),
                "vT": tr(v[b]),
                "wq": wcat(Wq, half),
                "wk": wcat(Wk, half),
                "wv": wcat(Wv, half),
                "wpT": np.ascontiguousarray(Wp[:, off : off + DC].T).astype(NPBF16),
                "pat": patflat,
                "selp2": _SELP2,
            }
        )
    return nc, in_maps


def kernel(k, q, v, mask, Wk, Wq, Wv, Wp, bp):
    bp = np.asarray(bp, np.float32)
    nc, in_maps = _prepare(k, q, v, mask, Wk, Wq, Wv, Wp)
    res = run_bass_kernel_spmd(nc, in_maps, list(range(8)))
    out = np.empty((B, T, E), np.float32)
    for b in range(B):
        yt = res.results[2 * b]["yT"] + res.results[2 * b + 1]["yT"]
        out[b] = yt.T + bp[None, :]
    return out


# revision 14
# speedup vs baseline: 1.0615x; 1.0615x over previous
"""Multi-head causal attention (B=4, T=2048, E=1024, H=16, D=64) on 8 trn2
NeuronCores via Bass/Tile.

Sharding: core c handles batch b = c//2 and heads [half*8, half*8+8), half =
c%2. Each core computes its 8 heads' attention and a partial output
projection Y^T = Wp_slice^T-contraction over its heads; the host sums the two
half partials per batch, transposes, and adds the bias.

On-device layout is "transposed": activations are [feature, token] so every
matmul contracts over the partition dim. Softmax denominators come from a
ones-column appended to the stationary V operand (M=65 matmuls); masking is
applied block-wise (128x128) with patterns derived from the actual mask input
at build time. No max-subtraction is needed: scores are ~N(0, 0.083^2).

Scheduling: the kernel is software-pipelined around the ACT-engine exp, which
is the per-block rate limiter during attention. Dense PE work (the next
t-tile's projections and the previous tile's output projection) is split into
single-matmul "filler" closures that are popped between attention i-groups to
fill what would otherwise be PE stalls. A dummy-matmul warmup at t=0 flips
the PE HAM clock gate to 8/8 before real work lands. Softmax normalization is
per head-pair: reciprocal_approx_fast on the psum row, then a K=1 float32r
broadcast matmul.
"""
import numpy as np
import ml_dtypes
from contextlib import ExitStack

import concourse.bass as bass
import concourse.mybir as mybir
import concourse.tile as tile
from concourse.bass_utils import run_bass_kernel_spmd
from concourse.vector_clock import ScopedClock

BF16 = mybir.dt.bfloat16
F32 = mybir.dt.float32
F32R = mybir.dt.float32r
NPBF16 = ml_dtypes.bfloat16

B, T, E, H, D = 4, 2048, 1024, 16, 64
HPC = 8            # heads per core
DC = HPC * D       # 512: stacked head dim per core
TJ = 512           # t tile (matmul free dim)
NJ = T // TJ       # 4
SI = 128           # s tile (psum partition dim)
NSI = T // SI      # 16
EC = E // 128      # 8 e-chunks
NP = HPC // 2      # 4 head pairs
_DUMMY_FILL = False
_DROP_OWN_WAITS = False

# ---------------------------------------------------------------------------
# Workarounds for this walrus build: at most ONE sync wait per instruction.
# ---------------------------------------------------------------------------
_PATCHED = False


def _patched_drain_and_barrier(self, tick_clock, wait_clock):
    drain_inst = self.nc.sync.drain(fusable=False)
    wait_clock.add_sem_waits(
        drain_inst.ins, ScopedClock({None: tick_clock.global_clock})
    )
    si = drain_inst.ins.sync_info
    if si is not None and len(si.on_wait) > 1:
        waits = list(si.on_wait)
        drain_inst.ins.sync_info = mybir.SyncInfo(
            on_wait=waits[:1], on_update=list(si.on_update)
        )
        for ofs in range(1, len(waits)):
            extra = self.nc.sync.drain(fusable=False)
            extra.ins.sync_info = mybir.SyncInfo(
                on_wait=waits[ofs : ofs + 1], on_update=[]
            )
    self.nc.all_engine_barrier()
    assert self.sems is not None
    popped = self.nc._tile_sem_poison_stack.pop()
    assert popped is self._sem_poison
    self.nc.clear_and_free_semaphores(list(self.sems.allocated().values()))
    self.nc.all_engine_barrier()


def _install_patches():
    global _PATCHED
    if _PATCHED:
        return
    tile.TileContext._drain_and_barrier = _patched_drain_and_barrier
    _PATCHED = True


def _make_carrier(nc, engine, wait):
    """Wait-only EventSemaphore on `engine` (cheap: ~70ns, no pipe flush)."""
    ev = mybir.InstEventSemaphore(name=f"W-{nc.next_id()}", ins=[], outs=[])
    ev.engine = engine
    ev.sync_info = mybir.SyncInfo(on_wait=[wait], on_update=[])
    return ev


_ENGINE_SEM = {
    "EngineType.PE": "PE",
    "EngineType.DVE": "DVE",
    "EngineType.Activation": "Activation",
    "EngineType.SP": "SP",
    "EngineType.Pool": "Pool",
}
# engines with in-order issue AND in-order completion for these inst types:
# a wait on the engine's own completion sem is redundant. Ldweights excluded
# (the PE reorder window pulls it ahead of in-flight matmuls).
_DROPPABLE = (
    "InstMatmult", "InstActivation", "InstTensorTensor", "InstTensorCopy",
    "InstTensorReduce", "InstMemset", "InstReciprocal", "InstDMACopy",
    "InstCopyPredicated", "InstTensorScalarPtr", "InstTensorScalar",
    "InstCast", "InstDveOp", "InstCustomDve",
)


def _split_multi_waits(nc):
    for bbw in list(nc.bb_map.values()):
        bb = bbw.bb
        insts = bb.instructions
        if not any(
            getattr(i, "sync_info", None) is not None and len(i.sync_info.on_wait) > 1
            for i in insts
        ):
            continue
        out = []
        for inst in insts:
            si = getattr(inst, "sync_info", None)
            waits = list(si.on_wait) if si is not None else []
            if len(waits) > 1 and _DROP_OWN_WAITS:
                own = _ENGINE_SEM.get(str(inst.engine))
                tn = type(inst).__name__
                if own is not None and tn.startswith(_DROPPABLE):
                    waits = [
                        w for w in waits
                        if w.ant_name.rsplit("_", 1)[0] != own
                    ] or waits[-1:]
            if len(waits) > 1:
                for w in waits[:-1]:
                    out.append(_make_carrier(nc, inst.engine, w))
                waits = waits[-1:]
            if si is not None and list(si.on_wait) != waits:
                inst.sync_info = mybir.SyncInfo(
                    on_wait=waits, on_update=list(si.on_update)
                )
            out.append(inst)
        insts[:] = out


# ---------------------------------------------------------------------------
# Mask analysis (host side, 128x128 blocks).
# ---------------------------------------------------------------------------
def _classify_mask(mask):
    """mask: [T, T] bool, mask[t, s]=True means masked (score -> -inf).

    Returns (btab, patterns): btab[i][jj] in {'skip', 'dense', int u};
    patterns[u] is a [128,128] bf16 multiplier in [s, t] orientation."""
    nb = T // 128
    m = np.asarray(mask, dtype=bool)
    patterns = []
    index = {}
    btab = [[None] * nb for _ in range(nb)]
    for i in range(nb):          # s block
        for jj in range(nb):     # t block
            sub = m[jj * 128 : (jj + 1) * 128, i * 128 : (i + 1) * 128]  # [t, s]
            if sub.all():
                btab[i][jj] = "skip"
            elif not sub.any():
                btab[i][jj] = "dense"
            else:
                pat = (~sub).T.astype(NPBF16)  # [s, t] multiplier
                key = pat.tobytes()
                if key not in index:
                    index[key] = len(patterns)
                    patterns.append(pat)
                btab[i][jj] = index[key]
    if not patterns:
        patterns.append(np.ones((128, 128), NPBF16))
    return btab, np.stack(patterns)


# ---------------------------------------------------------------------------
# Kernel builder (SPMD program, identical on all 8 cores).
# ---------------------------------------------------------------------------
def _build(btab, n_pat):
    nc = bass.Bass()
    qT = nc.declare_dram_parameter("qT", [E, T], BF16, isOutput=False)
    kT = nc.declare_dram_parameter("kT", [E, T], BF16, isOutput=False)
    vT = nc.declare_dram_parameter("vT", [E, T], BF16, isOutput=False)
    wq = nc.declare_dram_parameter("wq", [E, DC], BF16, isOutput=False)
    wk = nc.declare_dram_parameter("wk", [E, DC], BF16, isOutput=False)
    wv = nc.declare_dram_parameter("wv", [E, DC], BF16, isOutput=False)
    wpT = nc.declare_dram_parameter("wpT", [DC, E], BF16, isOutput=False)
    pat = nc.declare_dram_parameter("pat", [n_pat * 128, 128], BF16, isOutput=False)
    selp2 = nc.declare_dram_parameter("selp2", [2, 128], BF16, isOutput=False)
    yT = nc.declare_dram_parameter("yT", [E, T], F32, isOutput=True)

    with ExitStack() as ctx:
        tc = ctx.enter_context(tile.TileContext(nc))
        # SBUF pools
        consts = ctx.enter_context(tc.tile_pool(name="consts", bufs=1))
        streams = ctx.enter_context(tc.tile_pool(name="streams", bufs=1))
        acts = ctx.enter_context(tc.tile_pool(name="acts", bufs=1))
        work = ctx.enter_context(tc.tile_pool(name="work", bufs=1))
        # PSUM pools
        psA = ctx.enter_context(tc.tile_pool(name="psA", bufs=1, space="PSUM"))
        psB = ctx.enter_context(tc.tile_pool(name="psB", bufs=1, space="PSUM"))

        # ---- constants ----
        wq_sb = [consts.tile([128, DC], BF16, tag=f"wq{e}", name=f"wq{e}", bufs=1) for e in range(EC)]
        wk_sb = [consts.tile([128, DC], BF16, tag=f"wk{e}", name=f"wk{e}", bufs=1) for e in range(EC)]
        wv_sb = [consts.tile([128, DC], BF16, tag=f"wv{e}", name=f"wv{e}", bufs=1) for e in range(EC)]
        wp_sb = [consts.tile([128, E], BF16, tag=f"wp{p}", name=f"wp{p}", bufs=1) for p in range(NP)]
        pat_sb = [consts.tile([128, 128], BF16, tag=f"pat{u}", name=f"pat{u}", bufs=1) for u in range(n_pat)]
        selp2_sb = consts.tile([2, 128], BF16, tag="selp2", name="selp2", bufs=1)
        dummy_sb = consts.tile([128, TJ], BF16, tag="dummy", name="dummy", bufs=1)

        # ---- warmup: flip the PE HAM clock gate to 8/8 while DMAs land ----
        nc.vector.memset(dummy_sb[:], 0.0)
        warm_ps = psA.tile([128, TJ], F32, tag="mm512", bufs=2, name="warm")
        for _ in range(10):
            nc.tensor.matmul(
                warm_ps[:], dummy_sb[:, 0:128], dummy_sb[:], start=True, stop=True
            )
        warm_n = [0]

        def dummy_fill(n_mms, ncols=TJ):
            """Keep the PE busy/warm across a known stall with throwaway MMs."""
            if not _DUMMY_FILL:
                return
            warm_n[0] += 1
            ps = psA.tile([128, TJ], F32, tag="mm512", bufs=2,
                          name=f"warmf{warm_n[0]}")
            for _ in range(n_mms):
                nc.tensor.matmul(
                    ps[:, 0:ncols], dummy_sb[:, 0:128], dummy_sb[:, 0:ncols],
                    start=True, stop=True,
                )

        # ---- persistent activations ----
        xq_sb = [acts.tile([128, T], BF16, tag=f"xq{p}", name=f"xq{p}", bufs=1) for p in range(NP)]
        xk_sb = [acts.tile([128, T], BF16, tag=f"xk{p}", name=f"xk{p}", bufs=1) for p in range(NP)]
        # xv tiles: per s-tile, heads laid out as 8 x (64 cols xv | 1 col ones)
        xv_sb = [acts.tile([128, HPC * 65], BF16, tag=f"xv{i}", name=f"xv{i}", bufs=1) for i in range(NSI)]
        for i in range(NSI):
            nc.vector.memset(
                xv_sb[i][:].rearrange("p (h x) -> p h x", x=65)[:, :, 64:65], 1.0
            )
        osc_sb_all = [
            [acts.tile([128, TJ], BF16, tag=f"osc{p}_{jj}", name=f"osc{p}_{jj}", bufs=1)
             for p in range(NP)]
            for jj in range(2)
        ]

        EXP = mybir.ActivationFunctionType.Exp
        stream_tiles = {}

        def issue_dma(j):
            jt = slice(j * TJ, (j + 1) * TJ)
            qs = [streams.tile([128, TJ], BF16, tag=f"qs{e}", name=f"qs{e}_{j}", bufs=2) for e in range(EC)]
            ks = [streams.tile([128, TJ], BF16, tag=f"ks{e}", name=f"ks{e}_{j}", bufs=2) for e in range(EC)]
            vs = [streams.tile([128, TJ], BF16, tag=f"vs{e}", name=f"vs{e}_{j}", bufs=2) for e in range(EC)]
            for e in range(EC):
                er = slice(e * 128, (e + 1) * 128)
                nc.sync.dma_start(out=qs[e][:], in_=qT[er, jt])
                if j == 0:
                    nc.sync.dma_start(out=wq_sb[e][:], in_=wq[er, :])
            for e in range(EC):
                er = slice(e * 128, (e + 1) * 128)
                nc.sync.dma_start(out=ks[e][:], in_=kT[er, jt])
                if j == 0:
                    nc.sync.dma_start(out=wk_sb[e][:], in_=wk[er, :])
            for e in range(EC):
                er = slice(e * 128, (e + 1) * 128)
                nc.sync.dma_start(out=vs[e][:], in_=vT[er, jt])
                if j == 0:
                    nc.sync.dma_start(out=wv_sb[e][:], in_=wv[er, :])
            if j == 0:
                for u in range(n_pat):
                    nc.sync.dma_start(out=pat_sb[u][:], in_=pat[u * 128 : (u + 1) * 128, :])
                for p in range(NP):
                    nc.sync.dma_start(out=wp_sb[p][:], in_=wpT[p * 128 : (p + 1) * 128, :])
                nc.sync.dma_start(out=selp2_sb[:], in_=selp2[:])
            stream_tiles[j] = (qs, ks, vs)

        def proj_qk_fillers(j, pairs):
            """xq/xk projection for t-tile j, given pairs: one closure per MM."""
            qs, ks, _ = stream_tiles[j]
            jt = slice(j * TJ, (j + 1) * TJ)
            fillers = []
            for p in pairs:
                pc = slice(p * 128, (p + 1) * 128)
                for src, Wsb, dst in ((qs, wq_sb, xq_sb), (ks, wk_sb, xk_sb)):
                    cell = {}
                    for e in range(EC):
                        def f(cell=cell, src=src, Wsb=Wsb, dst=dst, e=e, p=p, pc=pc, jt=jt):
                            if e == 0:
                                cell["ps"] = psA.tile([128, TJ], F32, tag="mm512", bufs=2,
                                                      name=f"pqk_{j}_{p}")
                            nc.tensor.matmul(
                                cell["ps"][:], Wsb[e][:, pc],
                                src[e][:], start=(e == 0), stop=(e == EC - 1),
                            )
                            if e == EC - 1:
                                nc.vector.tensor_copy(dst[p][:, jt], cell["ps"][:])
                        fillers.append(f)
            return fillers

        def proj_v_fillers(j):
            """xv projection for t-tile j: one closure per MM."""
            _, _, vs = stream_tiles[j]
            fillers = []
            for loc in range(4):
                si = 4 * j + loc
                cell = {}
                for e in range(EC):
                    def f(cell=cell, e=e, loc=loc, si=si, vs=vs):
                        if e == 0:
                            cell["ps"] = psA.tile([128, DC], F32, tag="mm512", bufs=2,
                                                  name=f"pv_{si}")
                        nc.tensor.matmul(
                            cell["ps"][:], vs[e][:, loc * 128 : (loc + 1) * 128],
                            wv_sb[e][:], start=(e == 0), stop=(e == EC - 1),
                        )
                        if e == EC - 1:
                            nc.vector.tensor_copy(
                                xv_sb[si][:].rearrange("p (h x) -> p h x", x=65)[:, :, 0:64],
                                cell["ps"][:].rearrange("p (h d) -> p h d", h=HPC),
                            )
                    fillers.append(f)
            return fillers

        def y_fillers(j, osc_tiles):
            """output projection partial Y^T[:, j-tile]: one closure per MM."""
            jt = slice(j * TJ, (j + 1) * TJ)
            fillers = []
            for m in range(EC):
                cell = {}
                for p in range(NP):
                    def f(cell=cell, m=m, p=p, jt=jt, osc_tiles=osc_tiles, j=j):
                        if p == 0:
                            cell["ps"] = psA.tile([128, TJ], F32, tag="mm512", bufs=2,
                                                  name=f"y_{m}_{j}")
                        nc.tensor.matmul(
                            cell["ps"][:], wp_sb[p][:, m * 128 : (m + 1) * 128],
                            osc_tiles[p][:], start=(p == 0), stop=(p == NP - 1),
                        )
                        if p == NP - 1:
                            y_sb = work.tile([128, TJ], F32, tag="y", bufs=2,
                                             name=f"ysb_{m}_{j}")
                            nc.vector.tensor_copy(y_sb[:], cell["ps"][:])
                            nc.sync.dma_start(out=yT[m * 128 : (m + 1) * 128, jt], in_=y_sb[:])
                    fillers.append(f)
            return fillers

        def emit_av(j, p, o_ps, touched, ii, i, types, u, c0, n_i):
            """AV matmuls for s-block i of pair p (both heads)."""
            for hh in range(2):
                h = 2 * p + hh
                uo = hh * TJ
                runs = []  # (bl0, bl1, src_ap)
                bl = c0
                while bl < 4:
                    if types[bl] == "dense":
                        b2 = bl
                        while b2 + 1 < 4 and types[b2 + 1] == "dense":
                            b2 += 1
                        runs.append((bl, b2 + 1,
                                     u[:, uo + bl * 128 : uo + (b2 + 1) * 128]))
                        bl = b2 + 1
                    elif types[bl] == "skip":
                        bl += 1
                    else:
                        mt = work.tile([128, 128], BF16, tag="mfix", bufs=4)
                        nc.vector.tensor_mul(
                            mt[:], u[:, uo + bl * 128 : uo + (bl + 1) * 128],
                            pat_sb[types[bl]][:],
                        )
                        runs.append((bl, bl + 1, mt[:]))
                        bl += 1
                lhs_v = xv_sb[i][:, h * 65 : h * 65 + 65]
                for ri, (b0, b1, src) in enumerate(runs):
                    first = all(not touched[hh][b] for b in range(b0, b1))
                    assert first == any(
                        not touched[hh][b] for b in range(b0, b1)
                    ), "mask blocks: mixed touch state inside a run"
                    last = (ii == n_i - 1) and (ri == len(runs) - 1)
                    nc.tensor.matmul(
                        o_ps[hh][:, b0 * 128 : b1 * 128],
                        lhs_v, src,
                        start=first, stop=last,
                        skip_group_check=True,
                    )
                    for b in range(b0, b1):
                        touched[hh][b] = True

        def run_attention(j, filler_q):
            jt = slice(j * TJ, (j + 1) * TJ)
            osc_sb = osc_sb_all[j % 2]
            ivals = []
            for i in range(NSI):
                types = [btab[i][4 * j + bl] for bl in range(4)]
                if all(t == "skip" for t in types):
                    continue
                ivals.append((i, types))
            n_i = len(ivals)
            tails_out = []
            groups_total = max(1, NP * n_i)
            rate = len(filler_q) / groups_total
            state = {"acc": 0.0, "popped": 0, "g": 0}

            def pops(p):
                state["g"] += 1
                state["acc"] += rate
                if j == 0 and state["g"] <= 6:
                    return  # let the j=1 stream DMAs land first
                want = min(int(state["acc"]) - state["popped"], 3)
                if j == NJ - 1 and p == 0:
                    want = max(want, 2)
                for _ in range(want):
                    if filler_q:
                        filler_q.pop(0)()
                        state["popped"] += 1

            for p in range(NP):
                o_ps = [
                    psB.tile([65, TJ], F32, tag=f"ops{hh}", name=f"ops{hh}_{p}_{j}", bufs=1)
                    for hh in range(2)
                ]
                touched = [[False] * 4, [False] * 4]
                prev_chunk = []
                for ci in range(0, n_i, 2):
                    chunk = []
                    for ii in range(ci, min(ci + 2, n_i)):
                        i, types = ivals[ii]
                        c0 = next(bl for bl in range(4) if types[bl] != "skip")
                        # scores for both heads: row-tiled concurrent K=64 MMs
                        st = psA.tile([128, 2 * TJ], F32, tag="st", bufs=2)
                        for hh in range(2):
                            hr = slice(hh * 64, (hh + 1) * 64)
                            nc.tensor.matmul(
                                st[:, hh * TJ + c0 * 128 : (hh + 1) * TJ],
                                xk_sb[p][hr, i * 128 : (i + 1) * 128],
                                xq_sb[p][hr, jt][:, c0 * 128 : TJ],
                                start=True, stop=True,
                            )
                        u = work.tile([128, 2 * TJ], BF16, tag="u", bufs=4)
                        nc.scalar.activation(
                            u[:].rearrange("p (g c) -> p g c", g=2)[:, :, c0 * 128 : TJ],
                            st[:].rearrange("p (g c) -> p g c", g=2)[:, :, c0 * 128 : TJ],
                            EXP, scale=1.0 / 32.0,
                        )
                        chunk.append((ii, i, types, u, c0))
                    pops(p)
                    pops(p)
                    for g in prev_chunk:
                        emit_av(j, p, o_ps, touched, *g, n_i)
                    prev_chunk = chunk
                for g in prev_chunk:
                    emit_av(j, p, o_ps, touched, *g, n_i)

                # per-pair softmax denominators: scatter the two psum
                # ones-rows to [32, 32] (cheap 32-elem/lane reciprocal),
                # gather back, broadcast with a K=2 selector matmul.
                rcp2 = work.tile([32, 32], F32, tag="rcp2", bufs=4,
                                 name=f"rcp2_{p}_{j}")
                for hh in range(2):
                    rsb = work.tile([1, TJ], F32, tag="rsb", bufs=4)
                    nc.vector.tensor_copy(rsb[:], o_ps[hh][64:65, :])
                    nc.gpsimd.dma_start(
                        out=rcp2[16 * hh : 16 * hh + 16, :], in_=rsb[:]
                    )
                rrc2 = work.tile([32, 32], F32, tag="rrc2", bufs=4,
                                 name=f"rrc2_{p}_{j}")
                nc.vector.reciprocal(rrc2[:], rcp2[:])
                rrp = work.tile([2, TJ], BF16, tag="rrp", bufs=4,
                                name=f"rrp_{p}_{j}")
                nc.gpsimd.dma_start(out=rrp[:], in_=rrc2[:])
                osb = work.tile([128, TJ], BF16, tag="osb", bufs=8,
                                name=f"osb_{p}_{j}")
                for hh in range(2):
                    nc.vector.tensor_copy(
                        osb[hh * 64 : (hh + 1) * 64, :], o_ps[hh][0:64, :]
                    )

                def tail_p(p=p, rrp=rrp, osb=osb, osc=osc_sb[p], j=j):
                    rb_ps = psA.tile([128, TJ], F32, tag="mm512", bufs=2,
                                     name=f"rb_{p}_{j}")
                    nc.tensor.matmul(
                        rb_ps[:], selp2_sb[:], rrp[:], start=True, stop=True,
                    )
                    nc.vector.tensor_mul(osc[:], osb[:], rb_ps[:])

                if j == NJ - 1 and p == NP - 1:
                    last_tail[0] = tail_p
                elif j == NJ - 1:
                    filler_q.append(tail_p)
                else:
                    tails_out.append(tail_p)

            # drain leftovers
            while filler_q:
                filler_q.pop(0)()
            return tails_out

        # ------------------- main flow -------------------
        issue_dma(0)
        last_tail = [None]
        for idx, f in enumerate(proj_qk_fillers(0, range(NP)) + proj_v_fillers(0)):
            f()
            if idx % 8 == 7:
                dummy_fill(2, 256)

        pending = []
        deferred = []
        for j in range(NJ):
            if j + 1 < NJ:
                issue_dma(j + 1)
            filler_q = []
            filler_q += deferred
            deferred = []
            filler_q += pending
            if j + 1 < NJ:
                if j + 1 < NJ - 1:
                    filler_q += proj_qk_fillers(j + 1, range(NP))
                    filler_q += proj_v_fillers(j + 1)
                else:
                    # final tile: keep some projection work as filler for the
                    # filler-starved last attention phase
                    filler_q += proj_qk_fillers(j + 1, [0, 1])
                    deferred = proj_v_fillers(j + 1) + proj_qk_fillers(j + 1, [2, 3])
            tails = run_attention(j, filler_q)
            pending = tails + y_fillers(j, osc_sb_all[j % 2])
        dummy_fill(12)
        last_tail[0]()
        for f in pending:
            f()

    _split_multi_waits(nc)
    return nc


_SELP2 = np.zeros((2, 128), NPBF16)
_SELP2[0, 0:64] = 1.0
_SELP2[1, 64:128] = 1.0

_CACHE = {}


def _get_program(mask):
    key = np.asarray(mask, dtype=bool).tobytes()
    prog = _CACHE.get(key)
    if prog is None:
        _install_patches()
        btab, patterns = _classify_mask(mask)
        nc = _build(btab, len(patterns))
        prog = (nc, patterns)
        _CACHE[key] = prog
    return prog


def _prepare(k, q, v, mask, Wk, Wq, Wv, Wp):
    """Build (cached) the SPMD program and the 8 per-core input maps."""
    k = np.asarray(k, np.float32)
    q = np.asarray(q, np.float32)
    v = np.asarray(v, np.float32)
    Wk = np.asarray(Wk, np.float32)
    Wq = np.asarray(Wq, np.float32)
    Wv = np.asarray(Wv, np.float32)
    Wp = np.asarray(Wp, np.float32)

    nc, patterns = _get_program(mask)
    patflat = np.ascontiguousarray(patterns.reshape(-1, 128))

    def tr(x):  # [T, E] f32 -> [E, T] bf16 contiguous
        return np.ascontiguousarray(x.astype(NPBF16).T)

    def wcat(W, half):  # [H, E, D] -> [E, 512] bf16 for this half's 8 heads
        return np.ascontiguousarray(
            W[half * HPC : (half + 1) * HPC].transpose(1, 0, 2).reshape(E, DC)
        ).astype(NPBF16)

    in_maps = []
    for c in range(8):
        b, half = divmod(c, 2)
        off = half * DC
        in_maps.append(
            {
                "qT": tr(q[b]),
                "kT": tr(k[b]),
                "vT": tr(v[b]),
                "wq": wcat(Wq, half),
                "wk": wcat(Wk, half),
                "wv": wcat(Wv, half),
                "wpT": np.ascontiguousarray(Wp[:, off : off + DC].T).astype(NPBF16),
                "pat": patflat,
                "selp2": _SELP2,
            }
        )
    return nc, in_maps


def kernel(k, q, v, mask, Wk, Wq, Wv, Wp, bp):
    bp = np.asarray(bp, np.float32)
    nc, in_maps = _prepare(k, q, v, mask, Wk, Wq, Wv, Wp)
    res = run_bass_kernel_spmd(nc, in_maps, list(range(8)))
    out = np.empty((B, T, E), np.float32)
    for b in range(B):
        yt = res.results[2 * b]["yT"] + res.results[2 * b + 1]["yT"]
        out[b] = yt.T + bp[None, :]
    return out


# revision 15
# speedup vs baseline: 1.0820x; 1.0193x over previous
"""Multi-head causal attention (B=4, T=2048, E=1024, H=16, D=64) on 8 trn2
NeuronCores via Bass/Tile.

Sharding: core c handles batch b = c//2 and heads [half*8, half*8+8), half =
c%2. Each core computes its 8 heads' attention and a partial output
projection Y^T = Wp_slice^T-contraction over its heads; the host sums the two
half partials per batch, transposes, and adds the bias.

On-device layout is "transposed": activations are [feature, token] so every
matmul contracts over the partition dim. Softmax denominators come from a
ones-column appended to the stationary V operand (M=65 matmuls); masking is
applied block-wise (128x128) with patterns derived from the actual mask input
at build time. No max-subtraction is needed: scores are ~N(0, 0.083^2).

Scheduling: the kernel is software-pipelined around the ACT-engine exp, which
is the per-block rate limiter during attention. Dense PE work (the next
t-tile's projections and the previous tile's output projection) is split into
single-matmul "filler" closures that are popped between attention i-groups to
fill what would otherwise be PE stalls. A dummy-matmul warmup at t=0 flips
the PE HAM clock gate to 8/8 before real work lands. Softmax normalization is
per head-pair: reciprocal_approx_fast on the psum row, then a K=1 float32r
broadcast matmul.
"""
import numpy as np
import ml_dtypes
from contextlib import ExitStack

import concourse.bass as bass
import concourse.mybir as mybir
import concourse.tile as tile
from concourse.bass_utils import run_bass_kernel_spmd
from concourse.vector_clock import ScopedClock

BF16 = mybir.dt.bfloat16
F32 = mybir.dt.float32
F32R = mybir.dt.float32r
NPBF16 = ml_dtypes.bfloat16

B, T, E, H, D = 4, 2048, 1024, 16, 64
HPC = 8            # heads per core
DC = HPC * D       # 512: stacked head dim per core
TJ = 512           # t tile (matmul free dim)
NJ = T // TJ       # 4
SI = 128           # s tile (psum partition dim)
NSI = T // SI      # 16
EC = E // 128      # 8 e-chunks
NP = HPC // 2      # 4 head pairs
_DUMMY_FILL = True
_DROP_OWN_WAITS = False

# ---------------------------------------------------------------------------
# Workarounds for this walrus build: at most ONE sync wait per instruction.
# ---------------------------------------------------------------------------
_PATCHED = False


def _patched_drain_and_barrier(self, tick_clock, wait_clock):
    drain_inst = self.nc.sync.drain(fusable=False)
    wait_clock.add_sem_waits(
        drain_inst.ins, ScopedClock({None: tick_clock.global_clock})
    )
    si = drain_inst.ins.sync_info
    if si is not None and len(si.on_wait) > 1:
        waits = list(si.on_wait)
        drain_inst.ins.sync_info = mybir.SyncInfo(
            on_wait=waits[:1], on_update=list(si.on_update)
        )
        for ofs in range(1, len(waits)):
            extra = self.nc.sync.drain(fusable=False)
            extra.ins.sync_info = mybir.SyncInfo(
                on_wait=waits[ofs : ofs + 1], on_update=[]
            )
    self.nc.all_engine_barrier()
    assert self.sems is not None
    popped = self.nc._tile_sem_poison_stack.pop()
    assert popped is self._sem_poison
    self.nc.clear_and_free_semaphores(list(self.sems.allocated().values()))
    self.nc.all_engine_barrier()


def _install_patches():
    global _PATCHED
    if _PATCHED:
        return
    tile.TileContext._drain_and_barrier = _patched_drain_and_barrier
    _PATCHED = True


def _make_carrier(nc, engine, wait):
    """Wait-only EventSemaphore on `engine` (cheap: ~70ns, no pipe flush)."""
    ev = mybir.InstEventSemaphore(name=f"W-{nc.next_id()}", ins=[], outs=[])
    ev.engine = engine
    ev.sync_info = mybir.SyncInfo(on_wait=[wait], on_update=[])
    return ev


_ENGINE_SEM = {
    "EngineType.PE": "PE",
    "EngineType.DVE": "DVE",
    "EngineType.Activation": "Activation",
    "EngineType.SP": "SP",
    "EngineType.Pool": "Pool",
}
# engines with in-order issue AND in-order completion for these inst types:
# a wait on the engine's own completion sem is redundant. Ldweights excluded
# (the PE reorder window pulls it ahead of in-flight matmuls).
_DROPPABLE = (
    "InstMatmult", "InstActivation", "InstTensorTensor", "InstTensorCopy",
    "InstTensorReduce", "InstMemset", "InstReciprocal", "InstDMACopy",
    "InstCopyPredicated", "InstTensorScalarPtr", "InstTensorScalar",
    "InstCast", "InstDveOp", "InstCustomDve",
)


def _split_multi_waits(nc):
    for bbw in list(nc.bb_map.values()):
        bb = bbw.bb
        insts = bb.instructions
        if not any(
            getattr(i, "sync_info", None) is not None and len(i.sync_info.on_wait) > 1
            for i in insts
        ):
            continue
        out = []
        for inst in insts:
            si = getattr(inst, "sync_info", None)
            waits = list(si.on_wait) if si is not None else []
            if len(waits) > 1 and _DROP_OWN_WAITS:
                own = _ENGINE_SEM.get(str(inst.engine))
                tn = type(inst).__name__
                if own is not None and tn.startswith(_DROPPABLE):
                    waits = [
                        w for w in waits
                        if w.ant_name.rsplit("_", 1)[0] != own
                    ] or waits[-1:]
            if len(waits) > 1:
                for w in waits[:-1]:
                    out.append(_make_carrier(nc, inst.engine, w))
                waits = waits[-1:]
            if si is not None and list(si.on_wait) != waits:
                inst.sync_info = mybir.SyncInfo(
                    on_wait=waits, on_update=list(si.on_update)
                )
            out.append(inst)
        insts[:] = out


# ---------------------------------------------------------------------------
# Mask analysis (host side, 128x128 blocks).
# ---------------------------------------------------------------------------
def _classify_mask(mask):
    """mask: [T, T] bool, mask[t, s]=True means masked (score -> -inf).

    Returns (btab, patterns): btab[i][jj] in {'skip', 'dense', int u};
    patterns[u] is a [128,128] bf16 multiplier in [s, t] orientation."""
    nb = T // 128
    m = np.asarray(mask, dtype=bool)
    patterns = []
    index = {}
    btab = [[None] * nb for _ in range(nb)]
    for i in range(nb):          # s block
        for jj in range(nb):     # t block
            sub = m[jj * 128 : (jj + 1) * 128, i * 128 : (i + 1) * 128]  # [t, s]
            if sub.all():
                btab[i][jj] = "skip"
            elif not sub.any():
                btab[i][jj] = "dense"
            else:
                pat = (~sub).T.astype(NPBF16)  # [s, t] multiplier
                key = pat.tobytes()
                if key not in index:
                    index[key] = len(patterns)
                    patterns.append(pat)
                btab[i][jj] = index[key]
    if not patterns:
        patterns.append(np.ones((128, 128), NPBF16))
    return btab, np.stack(patterns)


# ---------------------------------------------------------------------------
# Kernel builder (SPMD program, identical on all 8 cores).
# ---------------------------------------------------------------------------
def _build(btab, n_pat):
    nc = bass.Bass()
    qT = nc.declare_dram_parameter("qT", [E, T], BF16, isOutput=False)
    kT = nc.declare_dram_parameter("kT", [E, T], BF16, isOutput=False)
    vT = nc.declare_dram_parameter("vT", [E, T], BF16, isOutput=False)
    wq = nc.declare_dram_parameter("wq", [E, DC], BF16, isOutput=False)
    wk = nc.declare_dram_parameter("wk", [E, DC], BF16, isOutput=False)
    wv = nc.declare_dram_parameter("wv", [E, DC], BF16, isOutput=False)
    wpT = nc.declare_dram_parameter("wpT", [DC, E], BF16, isOutput=False)
    pat = nc.declare_dram_parameter("pat", [n_pat * 128, 128], BF16, isOutput=False)
    selp2 = nc.declare_dram_parameter("selp2", [2, 128], BF16, isOutput=False)
    yT = nc.declare_dram_parameter("yT", [E, T], F32, isOutput=True)

    with ExitStack() as ctx:
        tc = ctx.enter_context(tile.TileContext(nc))
        # SBUF pools
        consts = ctx.enter_context(tc.tile_pool(name="consts", bufs=1))
        streams = ctx.enter_context(tc.tile_pool(name="streams", bufs=1))
        acts = ctx.enter_context(tc.tile_pool(name="acts", bufs=1))
        work = ctx.enter_context(tc.tile_pool(name="work", bufs=1))
        # PSUM pools
        psA = ctx.enter_context(tc.tile_pool(name="psA", bufs=1, space="PSUM"))
        psB = ctx.enter_context(tc.tile_pool(name="psB", bufs=1, space="PSUM"))

        # ---- constants ----
        wq_sb = [consts.tile([128, DC], BF16, tag=f"wq{e}", name=f"wq{e}", bufs=1) for e in range(EC)]
        wk_sb = [consts.tile([128, DC], BF16, tag=f"wk{e}", name=f"wk{e}", bufs=1) for e in range(EC)]
        wv_sb = [consts.tile([128, DC], BF16, tag=f"wv{e}", name=f"wv{e}", bufs=1) for e in range(EC)]
        wp_sb = [consts.tile([128, E], BF16, tag=f"wp{p}", name=f"wp{p}", bufs=1) for p in range(NP)]
        pat_sb = [consts.tile([128, 128], BF16, tag=f"pat{u}", name=f"pat{u}", bufs=1) for u in range(n_pat)]
        selp2_sb = consts.tile([2, 128], BF16, tag="selp2", name="selp2", bufs=1)
        dummy_sb = consts.tile([128, TJ], BF16, tag="dummy", name="dummy", bufs=1)

        # ---- warmup: flip the PE HAM clock gate to 8/8 while DMAs land ----
        nc.vector.memset(dummy_sb[:], 0.0)
        warm_ps = psA.tile([128, TJ], F32, tag="mm512", bufs=2, name="warm")
        for _ in range(10):
            nc.tensor.matmul(
                warm_ps[:], dummy_sb[:, 0:128], dummy_sb[:], start=True, stop=True
            )
        warm_n = [0]

        def dummy_fill(n_mms, ncols=TJ):
            """Keep the PE busy/warm across a known stall with throwaway MMs."""
            if not _DUMMY_FILL:
                return
            warm_n[0] += 1
            ps = psA.tile([128, TJ], F32, tag="mm512", bufs=2,
                          name=f"warmf{warm_n[0]}")
            for _ in range(n_mms):
                nc.tensor.matmul(
                    ps[:, 0:ncols], dummy_sb[:, 0:128], dummy_sb[:, 0:ncols],
                    start=True, stop=True,
                )

        # ---- persistent activations ----
        xq_sb = [acts.tile([128, T], BF16, tag=f"xq{p}", name=f"xq{p}", bufs=1) for p in range(NP)]
        xk_sb = [acts.tile([128, T], BF16, tag=f"xk{p}", name=f"xk{p}", bufs=1) for p in range(NP)]
        # xv tiles: per s-tile, heads laid out as 8 x (64 cols xv | 1 col ones)
        xv_sb = [acts.tile([128, HPC * 65], BF16, tag=f"xv{i}", name=f"xv{i}", bufs=1) for i in range(NSI)]
        for i in range(NSI):
            nc.vector.memset(
                xv_sb[i][:].rearrange("p (h x) -> p h x", x=65)[:, :, 64:65], 1.0
            )
        osc_sb_all = [
            [acts.tile([128, TJ], BF16, tag=f"osc{p}_{jj}", name=f"osc{p}_{jj}", bufs=1)
             for p in range(NP)]
            for jj in range(2)
        ]

        EXP = mybir.ActivationFunctionType.Exp
        stream_tiles = {}

        def issue_dma(j):
            jt = slice(j * TJ, (j + 1) * TJ)
            qs = [streams.tile([128, TJ], BF16, tag=f"qs{e}", name=f"qs{e}_{j}", bufs=2) for e in range(EC)]
            ks = [streams.tile([128, TJ], BF16, tag=f"ks{e}", name=f"ks{e}_{j}", bufs=2) for e in range(EC)]
            vs = [streams.tile([128, TJ], BF16, tag=f"vs{e}", name=f"vs{e}_{j}", bufs=2) for e in range(EC)]
            for e in range(EC):
                er = slice(e * 128, (e + 1) * 128)
                nc.sync.dma_start(out=qs[e][:], in_=qT[er, jt])
                if j == 0:
                    nc.sync.dma_start(out=wq_sb[e][:], in_=wq[er, :])
            for e in range(EC):
                er = slice(e * 128, (e + 1) * 128)
                nc.sync.dma_start(out=ks[e][:], in_=kT[er, jt])
                if j == 0:
                    nc.sync.dma_start(out=wk_sb[e][:], in_=wk[er, :])
            for e in range(EC):
                er = slice(e * 128, (e + 1) * 128)
                nc.sync.dma_start(out=vs[e][:], in_=vT[er, jt])
                if j == 0:
                    nc.sync.dma_start(out=wv_sb[e][:], in_=wv[er, :])
            if j == 0:
                for u in range(n_pat):
                    nc.sync.dma_start(out=pat_sb[u][:], in_=pat[u * 128 : (u + 1) * 128, :])
                for p in range(NP):
                    nc.sync.dma_start(out=wp_sb[p][:], in_=wpT[p * 128 : (p + 1) * 128, :])
                nc.sync.dma_start(out=selp2_sb[:], in_=selp2[:])
            stream_tiles[j] = (qs, ks, vs)

        def proj_qk_fillers(j, pairs):
            """xq/xk projection for t-tile j, given pairs: one closure per MM."""
            qs, ks, _ = stream_tiles[j]
            jt = slice(j * TJ, (j + 1) * TJ)
            fillers = []
            for p in pairs:
                pc = slice(p * 128, (p + 1) * 128)
                for src, Wsb, dst in ((qs, wq_sb, xq_sb), (ks, wk_sb, xk_sb)):
                    cell = {}
                    for e in range(EC):
                        def f(cell=cell, src=src, Wsb=Wsb, dst=dst, e=e, p=p, pc=pc, jt=jt):
                            if e == 0:
                                cell["ps"] = psA.tile([128, TJ], F32, tag="mm512", bufs=2,
                                                      name=f"pqk_{j}_{p}")
                            nc.tensor.matmul(
                                cell["ps"][:], Wsb[e][:, pc],
                                src[e][:], start=(e == 0), stop=(e == EC - 1),
                            )
                            if e == EC - 1:
                                nc.vector.tensor_copy(dst[p][:, jt], cell["ps"][:])
                        fillers.append(f)
            return fillers

        def proj_v_fillers(j):
            """xv projection for t-tile j: one closure per MM."""
            _, _, vs = stream_tiles[j]
            fillers = []
            for loc in range(4):
                si = 4 * j + loc
                cell = {}
                for e in range(EC):
                    def f(cell=cell, e=e, loc=loc, si=si, vs=vs):
                        if e == 0:
                            cell["ps"] = psA.tile([128, DC], F32, tag="mm512", bufs=2,
                                                  name=f"pv_{si}")
                        nc.tensor.matmul(
                            cell["ps"][:], vs[e][:, loc * 128 : (loc + 1) * 128],
                            wv_sb[e][:], start=(e == 0), stop=(e == EC - 1),
                        )
                        if e == EC - 1:
                            nc.vector.tensor_copy(
                                xv_sb[si][:].rearrange("p (h x) -> p h x", x=65)[:, :, 0:64],
                                cell["ps"][:].rearrange("p (h d) -> p h d", h=HPC),
                            )
                    fillers.append(f)
            return fillers

        def y_fillers(j, osc_tiles):
            """output projection partial Y^T[:, j-tile]: one closure per MM."""
            jt = slice(j * TJ, (j + 1) * TJ)
            fillers = []
            for m in range(EC):
                cell = {}
                for p in range(NP):
                    def f(cell=cell, m=m, p=p, jt=jt, osc_tiles=osc_tiles, j=j):
                        if p == 0:
                            cell["ps"] = psA.tile([128, TJ], F32, tag="mm512", bufs=2,
                                                  name=f"y_{m}_{j}")
                        nc.tensor.matmul(
                            cell["ps"][:], wp_sb[p][:, m * 128 : (m + 1) * 128],
                            osc_tiles[p][:], start=(p == 0), stop=(p == NP - 1),
                        )
                        if p == NP - 1:
                            y_sb = work.tile([128, TJ], F32, tag="y", bufs=4,
                                             name=f"ysb_{m}_{j}")
                            nc.vector.tensor_copy(y_sb[:], cell["ps"][:])
                            nc.sync.dma_start(out=yT[m * 128 : (m + 1) * 128, jt], in_=y_sb[:])
                    fillers.append(f)
            return fillers

        def emit_av(j, p, o_ps, touched, ii, i, types, u, c0, n_i):
            """AV matmuls for s-block i of pair p (both heads)."""
            for hh in range(2):
                h = 2 * p + hh
                uo = hh * TJ
                runs = []  # (bl0, bl1, src_ap)
                bl = c0
                while bl < 4:
                    if types[bl] == "dense":
                        b2 = bl
                        while b2 + 1 < 4 and types[b2 + 1] == "dense":
                            b2 += 1
                        runs.append((bl, b2 + 1,
                                     u[:, uo + bl * 128 : uo + (b2 + 1) * 128]))
                        bl = b2 + 1
                    elif types[bl] == "skip":
                        bl += 1
                    else:
                        mt = work.tile([128, 128], BF16, tag="mfix", bufs=4)
                        nc.vector.tensor_mul(
                            mt[:], u[:, uo + bl * 128 : uo + (bl + 1) * 128],
                            pat_sb[types[bl]][:],
                        )
                        runs.append((bl, bl + 1, mt[:]))
                        bl += 1
                lhs_v = xv_sb[i][:, h * 65 : h * 65 + 65]
                for ri, (b0, b1, src) in enumerate(runs):
                    first = all(not touched[hh][b] for b in range(b0, b1))
                    assert first == any(
                        not touched[hh][b] for b in range(b0, b1)
                    ), "mask blocks: mixed touch state inside a run"
                    last = (ii == n_i - 1) and (ri == len(runs) - 1)
                    nc.tensor.matmul(
                        o_ps[hh][:, b0 * 128 : b1 * 128],
                        lhs_v, src,
                        start=first, stop=last,
                        skip_group_check=True,
                    )
                    for b in range(b0, b1):
                        touched[hh][b] = True

        def run_attention(j, filler_q):
            jt = slice(j * TJ, (j + 1) * TJ)
            osc_sb = osc_sb_all[j % 2]
            ivals = []
            for i in range(NSI):
                types = [btab[i][4 * j + bl] for bl in range(4)]
                if all(t == "skip" for t in types):
                    continue
                ivals.append((i, types))
            n_i = len(ivals)
            tails_out = []
            groups_total = max(1, NP * n_i)
            rate = len(filler_q) / groups_total
            state = {"acc": 0.0, "popped": 0, "g": 0}

            def pops(p):
                state["g"] += 1
                state["acc"] += rate
                if j == 0 and state["g"] <= 6:
                    return  # let the j=1 stream DMAs land first
                want = min(int(state["acc"]) - state["popped"], 3)
                if j == NJ - 1 and p == 0:
                    want = max(want, 2)
                for _ in range(want):
                    if filler_q:
                        filler_q.pop(0)()
                        state["popped"] += 1

            for p in range(NP):
                o_ps = [
                    psB.tile([65, TJ], F32, tag=f"ops{hh}", name=f"ops{hh}_{p}_{j}", bufs=1)
                    for hh in range(2)
                ]
                touched = [[False] * 4, [False] * 4]
                prev_chunk = []
                for ci in range(0, n_i, 2):
                    chunk = []
                    for ii in range(ci, min(ci + 2, n_i)):
                        i, types = ivals[ii]
                        c0 = next(bl for bl in range(4) if types[bl] != "skip")
                        # scores for both heads: row-tiled concurrent K=64 MMs
                        st = psA.tile([128, 2 * TJ], F32, tag="st", bufs=2)
                        for hh in range(2):
                            hr = slice(hh * 64, (hh + 1) * 64)
                            nc.tensor.matmul(
                                st[:, hh * TJ + c0 * 128 : (hh + 1) * TJ],
                                xk_sb[p][hr, i * 128 : (i + 1) * 128],
                                xq_sb[p][hr, jt][:, c0 * 128 : TJ],
                                start=True, stop=True,
                            )
                        u = work.tile([128, 2 * TJ], BF16, tag="u", bufs=4)
                        nc.scalar.activation(
                            u[:].rearrange("p (g c) -> p g c", g=2)[:, :, c0 * 128 : TJ],
                            st[:].rearrange("p (g c) -> p g c", g=2)[:, :, c0 * 128 : TJ],
                            EXP, scale=1.0 / 32.0,
                        )
                        chunk.append((ii, i, types, u, c0))
                    pops(p)
                    pops(p)
                    for g in prev_chunk:
                        emit_av(j, p, o_ps, touched, *g, n_i)
                    prev_chunk = chunk
                for g in prev_chunk:
                    emit_av(j, p, o_ps, touched, *g, n_i)

                # per-pair softmax denominators: scatter the two psum
                # ones-rows to [32, 32] (cheap 32-elem/lane reciprocal),
                # gather back, broadcast with a K=2 selector matmul.
                final = j == NJ - 1 and p == NP - 1
                deng = nc.sync if final else nc.gpsimd
                rcp2 = work.tile([32, 32], F32, tag="rcp2", bufs=4,
                                 name=f"rcp2_{p}_{j}")
                for hh in range(2):
                    rsb = work.tile([1, TJ], F32, tag="rsb", bufs=4)
                    nc.vector.tensor_copy(rsb[:], o_ps[hh][64:65, :])
                    deng.dma_start(
                        out=rcp2[16 * hh : 16 * hh + 16, :], in_=rsb[:]
                    )
                rrc2 = work.tile([32, 32], F32, tag="rrc2", bufs=4,
                                 name=f"rrc2_{p}_{j}")
                nc.vector.reciprocal(rrc2[:], rcp2[:])
                rrp = work.tile([2, TJ], BF16, tag="rrp", bufs=4,
                                name=f"rrp_{p}_{j}")
                if final:
                    rrp32 = work.tile([2, TJ], F32, tag="rrp32", bufs=1,
                                      name=f"rrp32_{p}_{j}")
                    nc.sync.dma_start(out=rrp32[:], in_=rrc2[:])
                    nc.vector.tensor_copy(rrp[:], rrp32[:])
                else:
                    nc.gpsimd.dma_start(out=rrp[:], in_=rrc2[:])
                osb = work.tile([128, TJ], BF16, tag="osb", bufs=8,
                                name=f"osb_{p}_{j}")
                for hh in range(2):
                    nc.vector.tensor_copy(
                        osb[hh * 64 : (hh + 1) * 64, :], o_ps[hh][0:64, :]
                    )

                def tail_p(p=p, rrp=rrp, osb=osb, osc=osc_sb[p], j=j):
                    rb_ps = psA.tile([128, TJ], F32, tag="mm512", bufs=2,
                                     name=f"rb_{p}_{j}")
                    nc.tensor.matmul(
                        rb_ps[:], selp2_sb[:], rrp[:], start=True, stop=True,
                    )
                    nc.vector.tensor_mul(osc[:], osb[:], rb_ps[:])

                if j == NJ - 1 and p == NP - 1:
                    last_tail[0] = tail_p
                elif j == NJ - 1:
                    filler_q.append(tail_p)
                else:
                    tails_out.append(tail_p)

            # drain leftovers
            while filler_q:
                filler_q.pop(0)()
            return tails_out

        # ------------------- main flow -------------------
        issue_dma(0)
        last_tail = [None]
        for idx, f in enumerate(proj_qk_fillers(0, range(NP)) + proj_v_fillers(0)):
            f()
            if idx % 8 == 7:
                dummy_fill(2, 256)

        pending = []
        deferred = []
        for j in range(NJ):
            if j + 1 < NJ:
                issue_dma(j + 1)
            filler_q = []
            filler_q += deferred
            deferred = []
            filler_q += pending
            if j + 1 < NJ:
                if j + 1 < NJ - 1:
                    filler_q += proj_qk_fillers(j + 1, range(NP))
                    filler_q += proj_v_fillers(j + 1)
                else:
                    # final tile: keep some projection work as filler for the
                    # filler-starved last attention phase
                    filler_q += proj_qk_fillers(j + 1, [0, 1])
                    deferred = proj_v_fillers(j + 1) + proj_qk_fillers(j + 1, [2, 3])
            tails = run_attention(j, filler_q)
            pending = tails + y_fillers(j, osc_sb_all[j % 2])
        dummy_fill(26)
        last_tail[0]()
        for f in pending:
            f()

    _split_multi_waits(nc)
    return nc


_SELP2 = np.zeros((2, 128), NPBF16)
_SELP2[0, 0:64] = 1.0
_SELP2[1, 64:128] = 1.0

_CACHE = {}


def _get_program(mask):
    key = np.asarray(mask, dtype=bool).tobytes()
    prog = _CACHE.get(key)
    if prog is None:
        _install_patches()
        btab, patterns = _classify_mask(mask)
        nc = _build(btab, len(patterns))
        prog = (nc, patterns)
        _CACHE[key] = prog
    return prog


def _prepare(k, q, v, mask, Wk, Wq, Wv, Wp):
    """Build (cached) the SPMD program and the 8 per-core input maps."""
    k = np.asarray(k, np.float32)
    q = np.asarray(q, np.float32)
    v = np.asarray(v, np.float32)
    Wk = np.asarray(Wk, np.float32)
    Wq = np.asarray(Wq, np.float32)
    Wv = np.asarray(Wv, np.float32)
    Wp = np.asarray(Wp, np.float32)

    nc, patterns = _get_program(mask)
    patflat = np.ascontiguousarray(patterns.reshape(-1, 128))

    def tr(x):  # [T, E] f32 -> [E, T] bf16 contiguous
        return np.ascontiguousarray(x.astype(NPBF16).T)

    def wcat(W, half):  # [H, E, D] -> [E, 512] bf16 for this half's 8 heads
        return np.ascontiguousarray(
            W[half * HPC : (half + 1) * HPC].transpose(1, 0, 2).reshape(E, DC)
        ).astype(NPBF16)

    in_maps = []
    for c in range(8):
        b, half = divmod(c, 2)
        off = half * DC
        in_maps.append(
            {
                "qT": tr(q[b]),
                "kT": tr(k[b]),
                "vT": tr(v[b]),
                "wq": wcat(Wq, half),
                "wk": wcat(Wk, half),
                "wv": wcat(Wv, half),
                "wpT": np.ascontiguousarray(Wp[:, off : off + DC].T).astype(NPBF16),
                "pat": patflat,
                "selp2": _SELP2,
            }
        )
    return nc, in_maps


def kernel(k, q, v, mask, Wk, Wq, Wv, Wp, bp):
    bp = np.asarray(bp, np.float32)
    nc, in_maps = _prepare(k, q, v, mask, Wk, Wq, Wv, Wp)
    res = run_bass_kernel_spmd(nc, in_maps, list(range(8)))
    out = np.empty((B, T, E), np.float32)
    for b in range(B):
        yt = res.results[2 * b]["yT"] + res.results[2 * b + 1]["yT"]
        out[b] = yt.T + bp[None, :]
    return out


# revision 16
# speedup vs baseline: 1.0878x; 1.0053x over previous
"""Multi-head causal attention (B=4, T=2048, E=1024, H=16, D=64) on 8 trn2
NeuronCores via Bass/Tile.

Sharding: core c handles batch b = c//2 and heads [half*8, half*8+8), half =
c%2. Each core computes its 8 heads' attention and a partial output
projection Y^T = Wp_slice^T-contraction over its heads; the host sums the two
half partials per batch, transposes, and adds the bias.

On-device layout is "transposed": activations are [feature, token] so every
matmul contracts over the partition dim. Softmax denominators come from a
ones-column appended to the stationary V operand (M=65 matmuls); masking is
applied block-wise (128x128) with patterns derived from the actual mask input
at build time. No max-subtraction is needed: scores are ~N(0, 0.083^2).

Scheduling: the kernel is software-pipelined around the ACT-engine exp, which
is the per-block rate limiter during attention. Dense PE work (the next
t-tile's projections and the previous tile's output projection) is split into
single-matmul "filler" closures that are popped between attention i-groups to
fill what would otherwise be PE stalls. A dummy-matmul warmup at t=0 flips
the PE HAM clock gate to 8/8 before real work lands. Softmax normalization is
per head-pair: reciprocal_approx_fast on the psum row, then a K=1 float32r
broadcast matmul.
"""
import numpy as np
import ml_dtypes
from contextlib import ExitStack

import concourse.bass as bass
import concourse.mybir as mybir
import concourse.tile as tile
from concourse.bass_utils import run_bass_kernel_spmd
from concourse.vector_clock import ScopedClock

BF16 = mybir.dt.bfloat16
F32 = mybir.dt.float32
F32R = mybir.dt.float32r
NPBF16 = ml_dtypes.bfloat16

B, T, E, H, D = 4, 2048, 1024, 16, 64
HPC = 8            # heads per core
DC = HPC * D       # 512: stacked head dim per core
TJ = 512           # t tile (matmul free dim)
NJ = T // TJ       # 4
SI = 128           # s tile (psum partition dim)
NSI = T // SI      # 16
EC = E // 128      # 8 e-chunks
NP = HPC // 2      # 4 head pairs
_DUMMY_FILL = True
_DROP_OWN_WAITS = False

# ---------------------------------------------------------------------------
# Workarounds for this walrus build: at most ONE sync wait per instruction.
# ---------------------------------------------------------------------------
_PATCHED = False


def _patched_drain_and_barrier(self, tick_clock, wait_clock):
    drain_inst = self.nc.sync.drain(fusable=False)
    wait_clock.add_sem_waits(
        drain_inst.ins, ScopedClock({None: tick_clock.global_clock})
    )
    si = drain_inst.ins.sync_info
    if si is not None and len(si.on_wait) > 1:
        waits = list(si.on_wait)
        drain_inst.ins.sync_info = mybir.SyncInfo(
            on_wait=waits[:1], on_update=list(si.on_update)
        )
        for ofs in range(1, len(waits)):
            extra = self.nc.sync.drain(fusable=False)
            extra.ins.sync_info = mybir.SyncInfo(
                on_wait=waits[ofs : ofs + 1], on_update=[]
            )
    self.nc.all_engine_barrier()
    assert self.sems is not None
    popped = self.nc._tile_sem_poison_stack.pop()
    assert popped is self._sem_poison
    self.nc.clear_and_free_semaphores(list(self.sems.allocated().values()))
    self.nc.all_engine_barrier()


def _install_patches():
    global _PATCHED
    if _PATCHED:
        return
    tile.TileContext._drain_and_barrier = _patched_drain_and_barrier
    _PATCHED = True


def _make_carrier(nc, engine, wait):
    """Wait-only EventSemaphore on `engine` (cheap: ~70ns, no pipe flush)."""
    ev = mybir.InstEventSemaphore(name=f"W-{nc.next_id()}", ins=[], outs=[])
    ev.engine = engine
    ev.sync_info = mybir.SyncInfo(on_wait=[wait], on_update=[])
    return ev


_ENGINE_SEM = {
    "EngineType.PE": "PE",
    "EngineType.DVE": "DVE",
    "EngineType.Activation": "Activation",
    "EngineType.SP": "SP",
    "EngineType.Pool": "Pool",
}
# engines with in-order issue AND in-order completion for these inst types:
# a wait on the engine's own completion sem is redundant. Ldweights excluded
# (the PE reorder window pulls it ahead of in-flight matmuls).
_DROPPABLE = (
    "InstMatmult", "InstActivation", "InstTensorTensor", "InstTensorCopy",
    "InstTensorReduce", "InstMemset", "InstReciprocal", "InstDMACopy",
    "InstCopyPredicated", "InstTensorScalarPtr", "InstTensorScalar",
    "InstCast", "InstDveOp", "InstCustomDve",
)


def _split_multi_waits(nc):
    for bbw in list(nc.bb_map.values()):
        bb = bbw.bb
        insts = bb.instructions
        if not any(
            getattr(i, "sync_info", None) is not None and len(i.sync_info.on_wait) > 1
            for i in insts
        ):
            continue
        out = []
        for inst in insts:
            si = getattr(inst, "sync_info", None)
            waits = list(si.on_wait) if si is not None else []
            if len(waits) > 1 and _DROP_OWN_WAITS:
                own = _ENGINE_SEM.get(str(inst.engine))
                tn = type(inst).__name__
                if own is not None and tn.startswith(_DROPPABLE):
                    waits = [
                        w for w in waits
                        if w.ant_name.rsplit("_", 1)[0] != own
                    ] or waits[-1:]
            if len(waits) > 1:
                for w in waits[:-1]:
                    out.append(_make_carrier(nc, inst.engine, w))
                waits = waits[-1:]
            if si is not None and list(si.on_wait) != waits:
                inst.sync_info = mybir.SyncInfo(
                    on_wait=waits, on_update=list(si.on_update)
                )
            out.append(inst)
        insts[:] = out


# ---------------------------------------------------------------------------
# Mask analysis (host side, 128x128 blocks).
# ---------------------------------------------------------------------------
def _classify_mask(mask):
    """mask: [T, T] bool, mask[t, s]=True means masked (score -> -inf).

    Returns (btab, patterns): btab[i][jj] in {'skip', 'dense', int u};
    patterns[u] is a [128,128] bf16 multiplier in [s, t] orientation."""
    nb = T // 128
    m = np.asarray(mask, dtype=bool)
    patterns = []
    index = {}
    btab = [[None] * nb for _ in range(nb)]
    for i in range(nb):          # s block
        for jj in range(nb):     # t block
            sub = m[jj * 128 : (jj + 1) * 128, i * 128 : (i + 1) * 128]  # [t, s]
            if sub.all():
                btab[i][jj] = "skip"
            elif not sub.any():
                btab[i][jj] = "dense"
            else:
                pat = (~sub).T.astype(NPBF16)  # [s, t] multiplier
                key = pat.tobytes()
                if key not in index:
                    index[key] = len(patterns)
                    patterns.append(pat)
                btab[i][jj] = index[key]
    if not patterns:
        patterns.append(np.ones((128, 128), NPBF16))
    return btab, np.stack(patterns)


# ---------------------------------------------------------------------------
# Kernel builder (SPMD program, identical on all 8 cores).
# ---------------------------------------------------------------------------
def _build(btab, n_pat):
    nc = bass.Bass()
    qT = nc.declare_dram_parameter("qT", [E, T], BF16, isOutput=False)
    kT = nc.declare_dram_parameter("kT", [E, T], BF16, isOutput=False)
    vT = nc.declare_dram_parameter("vT", [E, T], BF16, isOutput=False)
    wq = nc.declare_dram_parameter("wq", [E, DC], BF16, isOutput=False)
    wk = nc.declare_dram_parameter("wk", [E, DC], BF16, isOutput=False)
    wv = nc.declare_dram_parameter("wv", [E, DC], BF16, isOutput=False)
    wpT = nc.declare_dram_parameter("wpT", [DC, E], BF16, isOutput=False)
    pat = nc.declare_dram_parameter("pat", [n_pat * 128, 128], BF16, isOutput=False)
    selp2 = nc.declare_dram_parameter("selp2", [2, 128], BF16, isOutput=False)
    yT = nc.declare_dram_parameter("yT", [E, T], F32, isOutput=True)

    with ExitStack() as ctx:
        tc = ctx.enter_context(tile.TileContext(nc))
        # SBUF pools
        consts = ctx.enter_context(tc.tile_pool(name="consts", bufs=1))
        streams = ctx.enter_context(tc.tile_pool(name="streams", bufs=1))
        acts = ctx.enter_context(tc.tile_pool(name="acts", bufs=1))
        work = ctx.enter_context(tc.tile_pool(name="work", bufs=1))
        # PSUM pools
        psA = ctx.enter_context(tc.tile_pool(name="psA", bufs=1, space="PSUM"))
        psB = ctx.enter_context(tc.tile_pool(name="psB", bufs=1, space="PSUM"))

        # ---- constants ----
        wq_sb = [consts.tile([128, DC], BF16, tag=f"wq{e}", name=f"wq{e}", bufs=1) for e in range(EC)]
        wk_sb = [consts.tile([128, DC], BF16, tag=f"wk{e}", name=f"wk{e}", bufs=1) for e in range(EC)]
        wv_sb = [consts.tile([128, DC], BF16, tag=f"wv{e}", name=f"wv{e}", bufs=1) for e in range(EC)]
        wp_sb = [consts.tile([128, E], BF16, tag=f"wp{p}", name=f"wp{p}", bufs=1) for p in range(NP)]
        pat_sb = [consts.tile([128, 128], BF16, tag=f"pat{u}", name=f"pat{u}", bufs=1) for u in range(n_pat)]
        selp2_sb = consts.tile([2, 128], BF16, tag="selp2", name="selp2", bufs=1)
        dummy_sb = consts.tile([128, TJ], BF16, tag="dummy", name="dummy", bufs=1)

        # ---- warmup: flip the PE HAM clock gate to 8/8 while DMAs land ----
        nc.vector.memset(dummy_sb[:], 0.0)
        warm_ps = psA.tile([128, TJ], F32, tag="mm512", bufs=2, name="warm")
        for _ in range(10):
            nc.tensor.matmul(
                warm_ps[:], dummy_sb[:, 0:128], dummy_sb[:], start=True, stop=True
            )
        warm_n = [0]

        def dummy_fill(n_mms, ncols=TJ):
            """Keep the PE busy/warm across a known stall with throwaway MMs."""
            if not _DUMMY_FILL:
                return
            warm_n[0] += 1
            ps = psA.tile([128, TJ], F32, tag="mm512", bufs=2,
                          name=f"warmf{warm_n[0]}")
            for _ in range(n_mms):
                nc.tensor.matmul(
                    ps[:, 0:ncols], dummy_sb[:, 0:128], dummy_sb[:, 0:ncols],
                    start=True, stop=True,
                )

        # ---- persistent activations ----
        xq_sb = [acts.tile([128, T], BF16, tag=f"xq{p}", name=f"xq{p}", bufs=1) for p in range(NP)]
        xk_sb = [acts.tile([128, T], BF16, tag=f"xk{p}", name=f"xk{p}", bufs=1) for p in range(NP)]
        # xv tiles: per s-tile, heads laid out as 8 x (64 cols xv | 1 col ones)
        xv_sb = [acts.tile([128, HPC * 65], BF16, tag=f"xv{i}", name=f"xv{i}", bufs=1) for i in range(NSI)]
        for i in range(NSI):
            nc.vector.memset(
                xv_sb[i][:].rearrange("p (h x) -> p h x", x=65)[:, :, 64:65], 1.0
            )
        osc_sb_all = [
            [acts.tile([128, TJ], BF16, tag=f"osc{p}_{jj}", name=f"osc{p}_{jj}", bufs=1)
             for p in range(NP)]
            for jj in range(2)
        ]

        EXP = mybir.ActivationFunctionType.Exp
        stream_tiles = {}

        def issue_dma(j):
            jt = slice(j * TJ, (j + 1) * TJ)
            qs = [streams.tile([128, TJ], BF16, tag=f"qs{e}", name=f"qs{e}_{j}", bufs=2) for e in range(EC)]
            ks = [streams.tile([128, TJ], BF16, tag=f"ks{e}", name=f"ks{e}_{j}", bufs=2) for e in range(EC)]
            vs = [streams.tile([128, TJ], BF16, tag=f"vs{e}", name=f"vs{e}_{j}", bufs=2) for e in range(EC)]
            for e in range(EC):
                er = slice(e * 128, (e + 1) * 128)
                nc.sync.dma_start(out=qs[e][:], in_=qT[er, jt])
                if j == 0:
                    nc.sync.dma_start(out=wq_sb[e][:], in_=wq[er, :])
            for e in range(EC):
                er = slice(e * 128, (e + 1) * 128)
                nc.sync.dma_start(out=ks[e][:], in_=kT[er, jt])
                if j == 0:
                    nc.sync.dma_start(out=wk_sb[e][:], in_=wk[er, :])
            for e in range(EC):
                er = slice(e * 128, (e + 1) * 128)
                nc.sync.dma_start(out=vs[e][:], in_=vT[er, jt])
                if j == 0:
                    nc.sync.dma_start(out=wv_sb[e][:], in_=wv[er, :])
            if j == 0:
                for u in range(n_pat):
                    nc.sync.dma_start(out=pat_sb[u][:], in_=pat[u * 128 : (u + 1) * 128, :])
                for p in range(NP):
                    nc.sync.dma_start(out=wp_sb[p][:], in_=wpT[p * 128 : (p + 1) * 128, :])
                nc.sync.dma_start(out=selp2_sb[:], in_=selp2[:])
            stream_tiles[j] = (qs, ks, vs)

        def proj_qk_fillers(j, pairs):
            """xq/xk projection for t-tile j, given pairs: one closure per MM."""
            qs, ks, _ = stream_tiles[j]
            jt = slice(j * TJ, (j + 1) * TJ)
            fillers = []
            for p in pairs:
                pc = slice(p * 128, (p + 1) * 128)
                for src, Wsb, dst in ((qs, wq_sb, xq_sb), (ks, wk_sb, xk_sb)):
                    cell = {}
                    for e in range(EC):
                        def f(cell=cell, src=src, Wsb=Wsb, dst=dst, e=e, p=p, pc=pc, jt=jt):
                            if e == 0:
                                cell["ps"] = psA.tile([128, TJ], F32, tag="mm512", bufs=2,
                                                      name=f"pqk_{j}_{p}")
                            nc.tensor.matmul(
                                cell["ps"][:], Wsb[e][:, pc],
                                src[e][:], start=(e == 0), stop=(e == EC - 1),
                            )
                            if e == EC - 1:
                                nc.vector.tensor_copy(dst[p][:, jt], cell["ps"][:])
                        fillers.append(f)
            return fillers

        def proj_v_fillers(j):
            """xv projection for t-tile j: one closure per MM."""
            _, _, vs = stream_tiles[j]
            fillers = []
            for loc in range(4):
                si = 4 * j + loc
                cell = {}
                for e in range(EC):
                    def f(cell=cell, e=e, loc=loc, si=si, vs=vs):
                        if e == 0:
                            cell["ps"] = psA.tile([128, DC], F32, tag="mm512", bufs=2,
                                                  name=f"pv_{si}")
                        nc.tensor.matmul(
                            cell["ps"][:], vs[e][:, loc * 128 : (loc + 1) * 128],
                            wv_sb[e][:], start=(e == 0), stop=(e == EC - 1),
                        )
                        if e == EC - 1:
                            nc.vector.tensor_copy(
                                xv_sb[si][:].rearrange("p (h x) -> p h x", x=65)[:, :, 0:64],
                                cell["ps"][:].rearrange("p (h d) -> p h d", h=HPC),
                            )
                    fillers.append(f)
            return fillers

        def y_fillers(j, osc_tiles):
            """output projection partial Y^T[:, j-tile]: one closure per MM."""
            jt = slice(j * TJ, (j + 1) * TJ)
            fillers = []
            for m in range(EC):
                cell = {}
                for p in range(NP):
                    def f(cell=cell, m=m, p=p, jt=jt, osc_tiles=osc_tiles, j=j):
                        if p == 0:
                            cell["ps"] = psA.tile([128, TJ], F32, tag="mm512", bufs=2,
                                                  name=f"y_{m}_{j}")
                        nc.tensor.matmul(
                            cell["ps"][:], wp_sb[p][:, m * 128 : (m + 1) * 128],
                            osc_tiles[p][:], start=(p == 0), stop=(p == NP - 1),
                        )
                        if p == NP - 1:
                            y_sb = work.tile([128, TJ], F32, tag="y", bufs=4,
                                             name=f"ysb_{m}_{j}")
                            nc.vector.tensor_copy(y_sb[:], cell["ps"][:])
                            nc.sync.dma_start(out=yT[m * 128 : (m + 1) * 128, jt], in_=y_sb[:])
                    fillers.append(f)
            return fillers

        def emit_av(j, p, o_ps, touched, ii, i, types, u, c0, n_i):
            """AV matmuls for s-block i of pair p (both heads).

            Mask patterns are applied in place on the exp output so each
            (i, head) needs exactly ONE matmul over the contiguous span."""
            assert all(t != "skip" for t in types[c0:4]), "interior skip block"
            for hh in range(2):
                h = 2 * p + hh
                uo = hh * TJ
                for bl in range(c0, 4):
                    if not isinstance(types[bl], str):
                        nc.vector.tensor_mul(
                            u[:, uo + bl * 128 : uo + (bl + 1) * 128],
                            u[:, uo + bl * 128 : uo + (bl + 1) * 128],
                            pat_sb[types[bl]][:],
                        )
                first = all(not touched[hh][b] for b in range(c0, 4))
                assert first == any(not touched[hh][b] for b in range(c0, 4))
                nc.tensor.matmul(
                    o_ps[hh][:, c0 * 128 : TJ],
                    xv_sb[i][:, h * 65 : h * 65 + 65],
                    u[:, uo + c0 * 128 : uo + TJ],
                    start=first, stop=(ii == n_i - 1),
                    skip_group_check=True,
                )
                for b in range(c0, 4):
                    touched[hh][b] = True

        def run_attention(j, filler_q):
            jt = slice(j * TJ, (j + 1) * TJ)
            osc_sb = osc_sb_all[j % 2]
            ivals = []
            for i in range(NSI):
                types = [btab[i][4 * j + bl] for bl in range(4)]
                if all(t == "skip" for t in types):
                    continue
                ivals.append((i, types))
            n_i = len(ivals)
            tails_out = []
            groups_total = max(1, NP * n_i)
            rate = len(filler_q) / groups_total
            state = {"acc": 0.0, "popped": 0, "g": 0}

            def pops(p):
                state["g"] += 1
                state["acc"] += rate
                if j == 0 and state["g"] <= 6:
                    return  # let the j=1 stream DMAs land first
                want = min(int(state["acc"]) - state["popped"], 3)
                if j == NJ - 1 and p == 0:
                    want = max(want, 2)
                for _ in range(want):
                    if filler_q:
                        filler_q.pop(0)()
                        state["popped"] += 1

            for p in range(NP):
                o_ps = [
                    psB.tile([65, TJ], F32, tag=f"ops{hh}", name=f"ops{hh}_{p}_{j}", bufs=1)
                    for hh in range(2)
                ]
                touched = [[False] * 4, [False] * 4]
                prev_chunk = []
                for ci in range(0, n_i, 2):
                    chunk = []
                    for ii in range(ci, min(ci + 2, n_i)):
                        i, types = ivals[ii]
                        c0 = next(bl for bl in range(4) if types[bl] != "skip")
                        # scores for both heads: row-tiled concurrent K=64 MMs
                        st = psA.tile([128, 2 * TJ], F32, tag="st", bufs=2)
                        for hh in range(2):
                            hr = slice(hh * 64, (hh + 1) * 64)
                            nc.tensor.matmul(
                                st[:, hh * TJ + c0 * 128 : (hh + 1) * TJ],
                                xk_sb[p][hr, i * 128 : (i + 1) * 128],
                                xq_sb[p][hr, jt][:, c0 * 128 : TJ],
                                start=True, stop=True,
                            )
                        u = work.tile([128, 2 * TJ], BF16, tag="u", bufs=6)
                        nc.scalar.activation(
                            u[:].rearrange("p (g c) -> p g c", g=2)[:, :, c0 * 128 : TJ],
                            st[:].rearrange("p (g c) -> p g c", g=2)[:, :, c0 * 128 : TJ],
                            EXP, scale=1.0 / 32.0,
                        )
                        chunk.append((ii, i, types, u, c0))
                    pops(p)
                    pops(p)
                    for g in prev_chunk:
                        emit_av(j, p, o_ps, touched, *g, n_i)
                    prev_chunk = chunk
                for g in prev_chunk:
                    emit_av(j, p, o_ps, touched, *g, n_i)

                # per-pair softmax denominators: scatter the two psum
                # ones-rows to [32, 32] (cheap 32-elem/lane reciprocal),
                # gather back, broadcast with a K=2 selector matmul.
                final = j == NJ - 1 and p == NP - 1
                deng = nc.sync if final else nc.gpsimd
                rcp2 = work.tile([32, 32], F32, tag="rcp2", bufs=4,
                                 name=f"rcp2_{p}_{j}")
                for hh in range(2):
                    rsb = work.tile([1, TJ], F32, tag="rsb", bufs=4)
                    nc.vector.tensor_copy(rsb[:], o_ps[hh][64:65, :])
                    deng.dma_start(
                        out=rcp2[16 * hh : 16 * hh + 16, :], in_=rsb[:]
                    )
                rrc2 = work.tile([32, 32], F32, tag="rrc2", bufs=4,
                                 name=f"rrc2_{p}_{j}")
                nc.vector.reciprocal(rrc2[:], rcp2[:])
                rrp = work.tile([2, TJ], BF16, tag="rrp", bufs=4,
                                name=f"rrp_{p}_{j}")
                if final:
                    rrp32 = work.tile([2, TJ], F32, tag="rrp32", bufs=1,
                                      name=f"rrp32_{p}_{j}")
                    nc.sync.dma_start(out=rrp32[:], in_=rrc2[:])
                    nc.vector.tensor_copy(rrp[:], rrp32[:])
                else:
                    nc.gpsimd.dma_start(out=rrp[:], in_=rrc2[:])
                osb = work.tile([128, TJ], BF16, tag="osb", bufs=8,
                                name=f"osb_{p}_{j}")
                for hh in range(2):
                    nc.vector.tensor_copy(
                        osb[hh * 64 : (hh + 1) * 64, :], o_ps[hh][0:64, :]
                    )

                def tail_p(p=p, rrp=rrp, osb=osb, osc=osc_sb[p], j=j):
                    rb_ps = psA.tile([128, TJ], F32, tag="mm512", bufs=2,
                                     name=f"rb_{p}_{j}")
                    nc.tensor.matmul(
                        rb_ps[:], selp2_sb[:], rrp[:], start=True, stop=True,
                    )
                    nc.vector.tensor_mul(osc[:], osb[:], rb_ps[:])

                if j == NJ - 1 and p == NP - 1:
                    last_tail[0] = tail_p
                elif j == NJ - 1:
                    filler_q.append(tail_p)
                else:
                    tails_out.append(tail_p)

            # drain leftovers
            while filler_q:
                filler_q.pop(0)()
            return tails_out

        # ------------------- main flow -------------------
        issue_dma(0)
        last_tail = [None]
        for idx, f in enumerate(proj_qk_fillers(0, range(NP)) + proj_v_fillers(0)):
            f()
            if idx % 8 == 7:
                dummy_fill(2, 256)

        pending = []
        deferred = []
        for j in range(NJ):
            if j + 1 < NJ:
                issue_dma(j + 1)
            filler_q = []
            filler_q += deferred
            deferred = []
            filler_q += pending
            if j + 1 < NJ:
                if j + 1 < NJ - 1:
                    filler_q += proj_qk_fillers(j + 1, range(NP))
                    filler_q += proj_v_fillers(j + 1)
                else:
                    # final tile: keep some projection work as filler for the
                    # filler-starved last attention phase
                    filler_q += proj_qk_fillers(j + 1, [0, 1])
                    deferred = proj_v_fillers(j + 1) + proj_qk_fillers(j + 1, [2, 3])
            tails = run_attention(j, filler_q)
            pending = tails + y_fillers(j, osc_sb_all[j % 2])
        dummy_fill(26)
        last_tail[0]()
        for f in pending:
            f()

    _split_multi_waits(nc)
    return nc


_SELP2 = np.zeros((2, 128), NPBF16)
_SELP2[0, 0:64] = 1.0
_SELP2[1, 64:128] = 1.0

_CACHE = {}


def _get_program(mask):
    key = np.asarray(mask, dtype=bool).tobytes()
    prog = _CACHE.get(key)
    if prog is None:
        _install_patches()
        btab, patterns = _classify_mask(mask)
        nc = _build(btab, len(patterns))
        prog = (nc, patterns)
        _CACHE[key] = prog
    return prog


def _prepare(k, q, v, mask, Wk, Wq, Wv, Wp):
    """Build (cached) the SPMD program and the 8 per-core input maps."""
    k = np.asarray(k, np.float32)
    q = np.asarray(q, np.float32)
    v = np.asarray(v, np.float32)
    Wk = np.asarray(Wk, np.float32)
    Wq = np.asarray(Wq, np.float32)
    Wv = np.asarray(Wv, np.float32)
    Wp = np.asarray(Wp, np.float32)

    nc, patterns = _get_program(mask)
    patflat = np.ascontiguousarray(patterns.reshape(-1, 128))

    def tr(x):  # [T, E] f32 -> [E, T] bf16 contiguous
        return np.ascontiguousarray(x.astype(NPBF16).T)

    def wcat(W, half):  # [H, E, D] -> [E, 512] bf16 for this half's 8 heads
        return np.ascontiguousarray(
            W[half * HPC : (half + 1) * HPC].transpose(1, 0, 2).reshape(E, DC)
        ).astype(NPBF16)

    in_maps = []
    for c in range(8):
        b, half = divmod(c, 2)
        off = half * DC
        in_maps.append(
            {
                "qT": tr(q[b]),
                "kT": tr(k[b]),
                "vT": tr(v[b]),
                "wq": wcat(Wq, half),
                "wk": wcat(Wk, half),
                "wv": wcat(Wv, half),
                "wpT": np.ascontiguousarray(Wp[:, off : off + DC].T).astype(NPBF16),
                "pat": patflat,
                "selp2": _SELP2,
            }
        )
    return nc, in_maps


def kernel(k, q, v, mask, Wk, Wq, Wv, Wp, bp):
    bp = np.asarray(bp, np.float32)
    nc, in_maps = _prepare(k, q, v, mask, Wk, Wq, Wv, Wp)
    res = run_bass_kernel_spmd(nc, in_maps, list(range(8)))
    out = np.empty((B, T, E), np.float32)
    for b in range(B):
        yt = res.results[2 * b]["yT"] + res.results[2 * b + 1]["yT"]
        out[b] = yt.T + bp[None, :]
    return out


# revision 20
# speedup vs baseline: 1.1161x; 1.0261x over previous
"""Multi-head causal attention (B=4, T=2048, E=1024, H=16, D=64) on 8 trn2
NeuronCores via Bass/Tile.

Sharding: core c handles batch b = c//2 and heads [half*8, half*8+8), half =
c%2. Each core computes its 8 heads' attention and a partial output
projection Y^T = Wp_slice^T-contraction over its heads; the host sums the two
half partials per batch, transposes, and adds the bias.

On-device layout is "transposed": activations are [feature, token] so every
matmul contracts over the partition dim. Softmax denominators come from a
ones-column appended to the stationary V operand (M=65 matmuls); masking is
applied block-wise (128x128) with patterns derived from the actual mask input
at build time. No max-subtraction is needed: scores are ~N(0, 0.083^2).

Scheduling: the kernel is software-pipelined around the ACT-engine exp, which
is the per-block rate limiter during attention. Dense PE work (the next
t-tile's projections and the previous tile's output projection) is split into
single-matmul "filler" closures that are popped between attention i-groups to
fill what would otherwise be PE stalls. A dummy-matmul warmup at t=0 flips
the PE HAM clock gate to 8/8 before real work lands. Softmax normalization is
per head-pair: reciprocal_approx_fast on the psum row, then a K=1 float32r
broadcast matmul.
"""
import numpy as np
import ml_dtypes
from contextlib import ExitStack

import concourse.bass as bass
import concourse.mybir as mybir
import concourse.tile as tile
from concourse.bass_utils import run_bass_kernel_spmd
from concourse.vector_clock import ScopedClock

BF16 = mybir.dt.bfloat16
F32 = mybir.dt.float32
F32R = mybir.dt.float32r
NPBF16 = ml_dtypes.bfloat16

B, T, E, H, D = 4, 2048, 1024, 16, 64
HPC = 8            # heads per core
DC = HPC * D       # 512: stacked head dim per core
TJ = 512           # t tile (matmul free dim)
NJ = T // TJ       # 4
SI = 128           # s tile (psum partition dim)
NSI = T // SI      # 16
EC = E // 128      # 8 e-chunks
NP = HPC // 2      # 4 head pairs
_DUMMY_FILL = True
_DROP_OWN_WAITS = False

# ---------------------------------------------------------------------------
# Workarounds for this walrus build: at most ONE sync wait per instruction.
# ---------------------------------------------------------------------------
_PATCHED = False


def _patched_drain_and_barrier(self, tick_clock, wait_clock):
    drain_inst = self.nc.sync.drain(fusable=False)
    wait_clock.add_sem_waits(
        drain_inst.ins, ScopedClock({None: tick_clock.global_clock})
    )
    si = drain_inst.ins.sync_info
    if si is not None and len(si.on_wait) > 1:
        waits = list(si.on_wait)
        drain_inst.ins.sync_info = mybir.SyncInfo(
            on_wait=waits[:1], on_update=list(si.on_update)
        )
        for ofs in range(1, len(waits)):
            extra = self.nc.sync.drain(fusable=False)
            extra.ins.sync_info = mybir.SyncInfo(
                on_wait=waits[ofs : ofs + 1], on_update=[]
            )
    self.nc.all_engine_barrier()
    assert self.sems is not None
    popped = self.nc._tile_sem_poison_stack.pop()
    assert popped is self._sem_poison
    self.nc.clear_and_free_semaphores(list(self.sems.allocated().values()))
    self.nc.all_engine_barrier()


def _install_patches():
    global _PATCHED
    if _PATCHED:
        return
    tile.TileContext._drain_and_barrier = _patched_drain_and_barrier
    _PATCHED = True


def _make_carrier(nc, engine, wait):
    """Wait-only EventSemaphore on `engine` (cheap: ~70ns, no pipe flush)."""
    ev = mybir.InstEventSemaphore(name=f"W-{nc.next_id()}", ins=[], outs=[])
    ev.engine = engine
    ev.sync_info = mybir.SyncInfo(on_wait=[wait], on_update=[])
    return ev


_ENGINE_SEM = {
    "EngineType.PE": "PE",
    "EngineType.DVE": "DVE",
    "EngineType.Activation": "Activation",
    "EngineType.SP": "SP",
    "EngineType.Pool": "Pool",
}
# engines with in-order issue AND in-order completion for these inst types:
# a wait on the engine's own completion sem is redundant. Ldweights excluded
# (the PE reorder window pulls it ahead of in-flight matmuls).
_DROPPABLE = (
    "InstMatmult", "InstActivation", "InstTensorTensor", "InstTensorCopy",
    "InstTensorReduce", "InstMemset", "InstReciprocal", "InstDMACopy",
    "InstCopyPredicated", "InstTensorScalarPtr", "InstTensorScalar",
    "InstCast", "InstDveOp", "InstCustomDve",
)


def _split_multi_waits(nc):
    for bbw in list(nc.bb_map.values()):
        bb = bbw.bb
        insts = bb.instructions
        if not any(
            getattr(i, "sync_info", None) is not None and len(i.sync_info.on_wait) > 1
            for i in insts
        ):
            continue
        out = []
        for inst in insts:
            si = getattr(inst, "sync_info", None)
            waits = list(si.on_wait) if si is not None else []
            if len(waits) > 1 and _DROP_OWN_WAITS:
                own = _ENGINE_SEM.get(str(inst.engine))
                tn = type(inst).__name__
                if own is not None and tn.startswith(_DROPPABLE):
                    waits = [
                        w for w in waits
                        if w.ant_name.rsplit("_", 1)[0] != own
                    ] or waits[-1:]
            if len(waits) > 1:
                for w in waits[:-1]:
                    out.append(_make_carrier(nc, inst.engine, w))
                waits = waits[-1:]
            if si is not None and list(si.on_wait) != waits:
                inst.sync_info = mybir.SyncInfo(
                    on_wait=waits, on_update=list(si.on_update)
                )
            out.append(inst)
        insts[:] = out


# ---------------------------------------------------------------------------
# Mask analysis (host side, 128x128 blocks).
# ---------------------------------------------------------------------------
def _classify_mask(mask):
    """mask: [T, T] bool, mask[t, s]=True means masked (score -> -inf).

    Returns (btab, patterns): btab[i][jj] in {'skip', 'dense', int u};
    patterns[u] is a [128,128] bf16 multiplier in [s, t] orientation."""
    nb = T // 128
    m = np.asarray(mask, dtype=bool)
    patterns = []
    index = {}
    btab = [[None] * nb for _ in range(nb)]
    for i in range(nb):          # s block
        for jj in range(nb):     # t block
            sub = m[jj * 128 : (jj + 1) * 128, i * 128 : (i + 1) * 128]  # [t, s]
            if sub.all():
                btab[i][jj] = "skip"
            elif not sub.any():
                btab[i][jj] = "dense"
            else:
                pat = (~sub).T.astype(NPBF16)  # [s, t] multiplier
                key = pat.tobytes()
                if key not in index:
                    index[key] = len(patterns)
                    patterns.append(pat)
                btab[i][jj] = index[key]
    if not patterns:
        patterns.append(np.ones((128, 128), NPBF16))
    return btab, np.stack(patterns)


# ---------------------------------------------------------------------------
# Kernel builder (SPMD program, identical on all 8 cores).
# ---------------------------------------------------------------------------
def _build(btab, n_pat):
    nc = bass.Bass()
    qT = nc.declare_dram_parameter("qT", [E, T], BF16, isOutput=False)
    kT = nc.declare_dram_parameter("kT", [E, T], BF16, isOutput=False)
    vT = nc.declare_dram_parameter("vT", [E, T], BF16, isOutput=False)
    wq = nc.declare_dram_parameter("wq", [E, DC], BF16, isOutput=False)
    wk = nc.declare_dram_parameter("wk", [E, DC], BF16, isOutput=False)
    wv = nc.declare_dram_parameter("wv", [E, DC], BF16, isOutput=False)
    wpT = nc.declare_dram_parameter("wpT", [DC, E], BF16, isOutput=False)
    pat = nc.declare_dram_parameter("pat", [n_pat * 128, 128], BF16, isOutput=False)
    selp2 = nc.declare_dram_parameter("selp2", [2, 128], BF16, isOutput=False)
    yT = nc.declare_dram_parameter("yT", [E, T], F32, isOutput=True)

    with ExitStack() as ctx:
        tc = ctx.enter_context(tile.TileContext(nc))
        # SBUF pools
        consts = ctx.enter_context(tc.tile_pool(name="consts", bufs=1))
        streams = ctx.enter_context(tc.tile_pool(name="streams", bufs=1))
        acts = ctx.enter_context(tc.tile_pool(name="acts", bufs=1))
        work = ctx.enter_context(tc.tile_pool(name="work", bufs=1))
        # PSUM pools
        psA = ctx.enter_context(tc.tile_pool(name="psA", bufs=1, space="PSUM"))
        psB = ctx.enter_context(tc.tile_pool(name="psB", bufs=1, space="PSUM"))

        # ---- constants ----
        wq_sb = [consts.tile([128, DC], BF16, tag=f"wq{e}", name=f"wq{e}", bufs=1) for e in range(EC)]
        wk_sb = [consts.tile([128, DC], BF16, tag=f"wk{e}", name=f"wk{e}", bufs=1) for e in range(EC)]
        wv_sb = [consts.tile([128, DC], BF16, tag=f"wv{e}", name=f"wv{e}", bufs=1) for e in range(EC)]
        wp_sb = [consts.tile([128, E], BF16, tag=f"wp{p}", name=f"wp{p}", bufs=1) for p in range(NP)]
        pat_sb = [consts.tile([128, 128], BF16, tag=f"pat{u}", name=f"pat{u}", bufs=1) for u in range(n_pat)]
        selp2_sb = consts.tile([2, 128], BF16, tag="selp2", name="selp2", bufs=1)
        dummy_sb = consts.tile([128, TJ], BF16, tag="dummy", name="dummy", bufs=1)

        # ---- warmup: flip the PE HAM clock gate to 8/8 while DMAs land ----
        nc.vector.memset(dummy_sb[:], 0.0)
        warm_ps = psA.tile([128, TJ], F32, tag="mm512", bufs=2, name="warm")
        for _ in range(10):
            nc.tensor.matmul(
                warm_ps[:], dummy_sb[:, 0:128], dummy_sb[:], start=True, stop=True
            )
        warm_n = [0]

        def dummy_fill(n_mms, ncols=TJ):
            """Keep the PE busy/warm across a known stall with throwaway MMs."""
            if not _DUMMY_FILL:
                return
            warm_n[0] += 1
            ps = psA.tile([128, TJ], F32, tag="mm512", bufs=2,
                          name=f"warmf{warm_n[0]}")
            for _ in range(n_mms):
                nc.tensor.matmul(
                    ps[:, 0:ncols], dummy_sb[:, 0:128], dummy_sb[:, 0:ncols],
                    start=True, stop=True,
                )

        # ---- persistent activations ----
        xq_sb = [acts.tile([128, T], BF16, tag=f"xq{p}", name=f"xq{p}", bufs=1) for p in range(NP)]
        xk_sb = [acts.tile([128, T], BF16, tag=f"xk{p}", name=f"xk{p}", bufs=1) for p in range(NP)]
        # xv tiles: per s-tile, heads laid out as 8 x (64 cols xv | 1 col ones)
        xv_sb = [acts.tile([128, HPC * 65], BF16, tag=f"xv{i}", name=f"xv{i}", bufs=1) for i in range(NSI)]
        for i in range(NSI):
            nc.vector.memset(
                xv_sb[i][:].rearrange("p (h x) -> p h x", x=65)[:, :, 64:65], 1.0
            )
        osc_sb_all = [
            [acts.tile([128, TJ], BF16, tag=f"osc{p}_{jj}", name=f"osc{p}_{jj}", bufs=1)
             for p in range(NP)]
            for jj in range(2)
        ]

        EXP = mybir.ActivationFunctionType.Exp
        stream_tiles = {}

        def issue_dma(j):
            jt = slice(j * TJ, (j + 1) * TJ)
            qs = [streams.tile([128, TJ], BF16, tag=f"qs{e}", name=f"qs{e}_{j}", bufs=2) for e in range(EC)]
            ks = [streams.tile([128, TJ], BF16, tag=f"ks{e}", name=f"ks{e}_{j}", bufs=2) for e in range(EC)]
            vs = [streams.tile([128, TJ], BF16, tag=f"vs{e}", name=f"vs{e}_{j}", bufs=2) for e in range(EC)]
            for e in range(EC):
                er = slice(e * 128, (e + 1) * 128)
                nc.sync.dma_start(out=qs[e][:], in_=qT[er, jt])
                if j == 0:
                    nc.sync.dma_start(out=wq_sb[e][:], in_=wq[er, :])
            for e in range(EC):
                er = slice(e * 128, (e + 1) * 128)
                nc.sync.dma_start(out=ks[e][:], in_=kT[er, jt])
                if j == 0:
                    nc.sync.dma_start(out=wk_sb[e][:], in_=wk[er, :])
            for e in range(EC):
                er = slice(e * 128, (e + 1) * 128)
                nc.sync.dma_start(out=vs[e][:], in_=vT[er, jt])
                if j == 0:
                    nc.sync.dma_start(out=wv_sb[e][:], in_=wv[er, :])
            if j == 0:
                for u in range(n_pat):
                    nc.sync.dma_start(out=pat_sb[u][:], in_=pat[u * 128 : (u + 1) * 128, :])
                for p in range(NP):
                    nc.sync.dma_start(out=wp_sb[p][:], in_=wpT[p * 128 : (p + 1) * 128, :])
                nc.sync.dma_start(out=selp2_sb[:], in_=selp2[:])
            stream_tiles[j] = (qs, ks, vs)

        def proj_qk_fillers(j, pairs):
            """xq/xk projection for t-tile j, given pairs: one closure per MM."""
            qs, ks, _ = stream_tiles[j]
            jt = slice(j * TJ, (j + 1) * TJ)
            fillers = []
            for p in pairs:
                pc = slice(p * 128, (p + 1) * 128)
                for src, Wsb, dst in ((qs, wq_sb, xq_sb), (ks, wk_sb, xk_sb)):
                    cell = {}
                    for e in range(EC):
                        def f(cell=cell, src=src, Wsb=Wsb, dst=dst, e=e, p=p, pc=pc, jt=jt):
                            if e == 0:
                                cell["ps"] = psA.tile([128, TJ], F32, tag="mm512", bufs=2,
                                                      name=f"pqk_{j}_{p}")
                            nc.tensor.matmul(
                                cell["ps"][:], Wsb[e][:, pc],
                                src[e][:], start=(e == 0), stop=(e == EC - 1),
                            )
                            if e == EC - 1:
                                nc.vector.tensor_copy(dst[p][:, jt], cell["ps"][:])
                        fillers.append(f)
            return fillers

        def proj_v_fillers(j):
            """xv projection for t-tile j: one closure per MM."""
            _, _, vs = stream_tiles[j]
            fillers = []
            for loc in range(4):
                si = 4 * j + loc
                cell = {}
                for e in range(EC):
                    def f(cell=cell, e=e, loc=loc, si=si, vs=vs):
                        if e == 0:
                            cell["ps"] = psA.tile([128, DC], F32, tag="mm512", bufs=2,
                                                  name=f"pv_{si}")
                        nc.tensor.matmul(
                            cell["ps"][:], vs[e][:, loc * 128 : (loc + 1) * 128],
                            wv_sb[e][:], start=(e == 0), stop=(e == EC - 1),
                        )
                        if e == EC - 1:
                            nc.vector.tensor_copy(
                                xv_sb[si][:].rearrange("p (h x) -> p h x", x=65)[:, :, 0:64],
                                cell["ps"][:].rearrange("p (h d) -> p h d", h=HPC),
                            )
                    fillers.append(f)
            return fillers

        def y_fillers(j, osc_tiles, pairs=None, acc_tiles=None, add_tiles=None):
            """output projection partial Y^T[:, j-tile]: one closure per MM.

            pairs: which head pairs to contract (default all). acc_tiles: if
            given, stage the psum into these SBUF tiles instead of DMA-ing
            out. add_tiles: if given, fuse-add these SBUF tiles into the
            result before the output DMA."""
            jt = slice(j * TJ, (j + 1) * TJ)
            if pairs is None:
                pairs = list(range(NP))
            fillers = []
            for m in range(EC):
                cell = {}
                for pi, p in enumerate(pairs):
                    def f(cell=cell, m=m, p=p, pi=pi, jt=jt, osc_tiles=osc_tiles, j=j):
                        if pi == 0:
                            cell["ps"] = psA.tile([128, TJ], F32, tag="mm512", bufs=2,
                                                  name=f"y_{m}_{j}_{p}")
                        nc.tensor.matmul(
                            cell["ps"][:], wp_sb[p][:, m * 128 : (m + 1) * 128],
                            osc_tiles[p][:], start=(pi == 0), stop=(pi == len(pairs) - 1),
                        )
                        if pi == len(pairs) - 1:
                            if acc_tiles is not None:
                                nc.vector.tensor_copy(acc_tiles[m][:], cell["ps"][:])
                                return
                            y_sb = work.tile([128, TJ], F32, tag="y", bufs=6,
                                             name=f"ysb_{m}_{j}")
                            if add_tiles is not None:
                                nc.vector.tensor_add(y_sb[:], cell["ps"][:], add_tiles[m][:])
                            else:
                                nc.vector.tensor_copy(y_sb[:], cell["ps"][:])
                            nc.sync.dma_start(out=yT[m * 128 : (m + 1) * 128, jt], in_=y_sb[:])
                    fillers.append(f)
            return fillers

        def emit_av(j, p, o_ps, touched, ii, i, types, u, c0, n_i):
            """AV matmuls for s-block i of pair p (both heads).

            Mask patterns are applied in place on the exp output so each
            (i, head) needs exactly ONE matmul over the contiguous span."""
            assert all(t != "skip" for t in types[c0:4]), "interior skip block"
            for hh in range(2):
                h = 2 * p + hh
                uo = hh * TJ
                for bl in range(c0, 4):
                    if not isinstance(types[bl], str):
                        nc.vector.tensor_mul(
                            u[:, uo + bl * 128 : uo + (bl + 1) * 128],
                            u[:, uo + bl * 128 : uo + (bl + 1) * 128],
                            pat_sb[types[bl]][:],
                        )
                first = all(not touched[hh][b] for b in range(c0, 4))
                assert first == any(not touched[hh][b] for b in range(c0, 4))
                nc.tensor.matmul(
                    o_ps[hh][:, c0 * 128 : TJ],
                    xv_sb[i][:, h * 65 : h * 65 + 65],
                    u[:, uo + c0 * 128 : uo + TJ],
                    start=first, stop=(ii == n_i - 1),
                    skip_group_check=True,
                )
                for b in range(c0, 4):
                    touched[hh][b] = True

        def run_attention(j, filler_q):
            jt = slice(j * TJ, (j + 1) * TJ)
            osc_sb = osc_sb_all[j % 2]
            ivals = []
            for i in range(NSI):
                types = [btab[i][4 * j + bl] for bl in range(4)]
                if all(t == "skip" for t in types):
                    continue
                ivals.append((i, types))
            n_i = len(ivals)
            tails_out = []
            groups_total = max(1, NP * n_i)
            rate = len(filler_q) / groups_total
            state = {"acc": 0.0, "popped": 0, "g": 0}

            def pops(p):
                state["g"] += 1
                state["acc"] += rate
                if j == 0 and state["g"] <= 6:
                    return  # let the j=1 stream DMAs land first
                cap = 3 if j == 0 else 4
                want = min(int(state["acc"]) - state["popped"], cap)
                if j == NJ - 1 and p == 0:
                    want = max(want, 2)
                for _ in range(want):
                    if filler_q:
                        filler_q.pop(0)()
                        state["popped"] += 1

            for p in range(NP):
                o_ps = [
                    psB.tile([65, TJ], F32, tag=f"ops{hh}", name=f"ops{hh}_{p}_{j}", bufs=1)
                    for hh in range(2)
                ]
                touched = [[False] * 4, [False] * 4]
                prev_chunk = []
                for ci in range(0, n_i, 2):
                    chunk = []
                    for ii in range(ci, min(ci + 2, n_i)):
                        i, types = ivals[ii]
                        c0 = next(bl for bl in range(4) if types[bl] != "skip")
                        # scores for both heads: row-tiled concurrent K=64 MMs
                        st = psA.tile([128, 2 * TJ], F32, tag="st", bufs=2)
                        for hh in range(2):
                            hr = slice(hh * 64, (hh + 1) * 64)
                            nc.tensor.matmul(
                                st[:, hh * TJ + c0 * 128 : (hh + 1) * TJ],
                                xk_sb[p][hr, i * 128 : (i + 1) * 128],
                                xq_sb[p][hr, jt][:, c0 * 128 : TJ],
                                start=True, stop=True,
                            )
                        u = work.tile([128, 2 * TJ], BF16, tag="u", bufs=6)
                        nc.scalar.activation(
                            u[:].rearrange("p (g c) -> p g c", g=2)[:, :, c0 * 128 : TJ],
                            st[:].rearrange("p (g c) -> p g c", g=2)[:, :, c0 * 128 : TJ],
                            EXP, scale=1.0 / 32.0,
                        )
                        chunk.append((ii, i, types, u, c0))
                    pops(p)
                    pops(p)
                    for g in prev_chunk:
                        emit_av(j, p, o_ps, touched, *g, n_i)
                    prev_chunk = chunk
                for g in prev_chunk:
                    emit_av(j, p, o_ps, touched, *g, n_i)

                # per-pair softmax denominators: scatter the two psum
                # ones-rows to [32, 32] (cheap 32-elem/lane reciprocal),
                # gather back, broadcast with a K=2 selector matmul.
                final = j == NJ - 1 and p == NP - 1
                deng = nc.sync if final else nc.gpsimd
                rcp2 = work.tile([32, 32], F32, tag="rcp2", bufs=8,
                                 name=f"rcp2_{p}_{j}")
                for hh in range(2):
                    rsb = work.tile([1, TJ], F32, tag="rsb", bufs=4)
                    nc.vector.tensor_copy(rsb[:], o_ps[hh][64:65, :])
                    deng.dma_start(
                        out=rcp2[16 * hh : 16 * hh + 16, :], in_=rsb[:]
                    )
                rrc2 = work.tile([32, 32], F32, tag="rrc2", bufs=8,
                                 name=f"rrc2_{p}_{j}")
                nc.vector.reciprocal(rrc2[:], rcp2[:])
                rrp = work.tile([2, TJ], BF16, tag="rrp", bufs=8,
                                name=f"rrp_{p}_{j}")
                if final:
                    rrp32 = work.tile([2, TJ], F32, tag="rrp32", bufs=1,
                                      name=f"rrp32_{p}_{j}")
                    nc.sync.dma_start(out=rrp32[:], in_=rrc2[:])
                    nc.vector.tensor_copy(rrp[:], rrp32[:])
                else:
                    nc.gpsimd.dma_start(out=rrp[:], in_=rrc2[:])
                osb = work.tile([128, TJ], BF16, tag="osb", bufs=6,
                                name=f"osb_{p}_{j}")
                for hh in range(2):
                    nc.vector.tensor_copy(
                        osb[hh * 64 : (hh + 1) * 64, :], o_ps[hh][0:64, :]
                    )

                def tail_p(p=p, rrp=rrp, osb=osb, osc=osc_sb[p], j=j):
                    rb_ps = psA.tile([128, TJ], F32, tag="mm512", bufs=2,
                                     name=f"rb_{p}_{j}")
                    nc.tensor.matmul(
                        rb_ps[:], selp2_sb[:], rrp[:], start=True, stop=True,
                    )
                    nc.vector.tensor_mul(osc[:], osb[:], rb_ps[:])

                if j == NJ - 1 and p == NP - 1:
                    last_tail[0] = tail_p
                elif j == NJ - 1:
                    filler_q.insert(min(12, len(filler_q)), tail_p)
                else:
                    tails_out.append(tail_p)

            # drain leftovers
            while filler_q:
                filler_q.pop(0)()
            return tails_out

        # ------------------- main flow -------------------
        issue_dma(0)
        last_tail = [None]
        for idx, f in enumerate(proj_qk_fillers(0, range(NP)) + proj_v_fillers(0)):
            f()
            if idx % 8 == 7:
                dummy_fill(2, 256)

        pending = []
        deferred = []
        for j in range(NJ):
            if j + 1 < NJ:
                issue_dma(j + 1)
            filler_q = []
            filler_q += deferred
            deferred = []
            filler_q += pending
            if j + 1 < NJ:
                if j + 1 < NJ - 1:
                    filler_q += proj_qk_fillers(j + 1, range(NP))
                    filler_q += proj_v_fillers(j + 1)
                else:
                    # final tile: keep some projection work as filler for the
                    # filler-starved last attention phase
                    filler_q += proj_qk_fillers(j + 1, [0, 1])
                    deferred = proj_v_fillers(j + 1) + proj_qk_fillers(j + 1, [2, 3])
            if j == NJ - 1:
                y3acc = [
                    work.tile([128, TJ], F32, tag="y3acc", bufs=8, name=f"y3a_{m}")
                    for m in range(EC)
                ]
                filler_q += y_fillers(j, osc_sb_all[j % 2], pairs=[0, 1],
                                      acc_tiles=y3acc)
            tails = run_attention(j, filler_q)
            pending = tails + (
                y_fillers(j, osc_sb_all[j % 2]) if j < NJ - 1 else []
            )
        dummy_fill(26)
        last_tail[0]()
        for f in y_fillers(NJ - 1, osc_sb_all[(NJ - 1) % 2], pairs=[2, 3],
                           add_tiles=y3acc):
            f()

    _split_multi_waits(nc)
    return nc


_SELP2 = np.zeros((2, 128), NPBF16)
_SELP2[0, 0:64] = 1.0
_SELP2[1, 64:128] = 1.0

_CACHE = {}


def _get_program(mask):
    key = np.asarray(mask, dtype=bool).tobytes()
    prog = _CACHE.get(key)
    if prog is None:
        _install_patches()
        btab, patterns = _classify_mask(mask)
        nc = _build(btab, len(patterns))
        prog = (nc, patterns)
        _CACHE[key] = prog
    return prog


def _prepare(k, q, v, mask, Wk, Wq, Wv, Wp):
    """Build (cached) the SPMD program and the 8 per-core input maps."""
    k = np.asarray(k, np.float32)
    q = np.asarray(q, np.float32)
    v = np.asarray(v, np.float32)
    Wk = np.asarray(Wk, np.float32)
    Wq = np.asarray(Wq, np.float32)
    Wv = np.asarray(Wv, np.float32)
    Wp = np.asarray(Wp, np.float32)

    nc, patterns = _get_program(mask)
    patflat = np.ascontiguousarray(patterns.reshape(-1, 128))

    def tr(x):  # [T, E] f32 -> [E, T] bf16 contiguous
        return np.ascontiguousarray(x.astype(NPBF16).T)

    def wcat(W, half):  # [H, E, D] -> [E, 512] bf16 for this half's 8 heads
        return np.ascontiguousarray(
            W[half * HPC : (half + 1) * HPC].transpose(1, 0, 2).reshape(E, DC)
        ).astype(NPBF16)

    in_maps = []
    for c in range(8):
        b, half = divmod(c, 2)
        off = half * DC
        in_maps.append(
            {
                "qT": tr(q[b]),
                "kT": tr(k[b]),
                "vT": tr(v[b]),
                "wq": wcat(Wq, half),
                "wk": wcat(Wk, half),
                "wv": wcat(Wv, half),
                "wpT": np.ascontiguousarray(Wp[:, off : off + DC].T).astype(NPBF16),
                "pat": patflat,
                "selp2": _SELP2,
            }
        )
    return nc, in_maps


def kernel(k, q, v, mask, Wk, Wq, Wv, Wp, bp):
    bp = np.asarray(bp, np.float32)
    nc, in_maps = _prepare(k, q, v, mask, Wk, Wq, Wv, Wp)
    res = run_bass_kernel_spmd(nc, in_maps, list(range(8)))
    out = np.empty((B, T, E), np.float32)
    for b in range(B):
        yt = res.results[2 * b]["yT"] + res.results[2 * b + 1]["yT"]
        out[b] = yt.T + bp[None, :]
    return out
